# revision 27
# baseline (speedup 1.0000x reference)
"""Trainium2 Bass kernel for 16-head causal MultiHeadAttention.

Problem: B=2, S=2048, D=1024, H=16 (head_dim 64), causal mask, f32 I/O.

Sharding (8 cores): core c handles batch b = c//4 and head-block hb = c%4
(4 heads = 256 embedding channels). Q/K/V projections are tensor-parallel
column slices; the output projection is tensor-parallel over rows of Wo.T,
so each core emits a partial (S, D) output that the host sums per batch.

Per-core kernel (all matmuls bf16, f32 accumulate):
  1. QT/KT = (Wq_c.T).T @ xT (+bias)   -> (256, 2048) SBUF, e in partitions
  2. V    = xT.T @ Wv_c.T              -> (2048, 256) "V_aug" layout with a
     ones column per head (for softmax row sums)
  3. scoresT[k, q] = KT.T @ QT per 128-key block (both heads of a pair per
     pass), causal blocks only; exp on ScalarE (scale=1/8, no max
     subtraction -- scores are O(5) so exp is safe in f32); the diagonal
     block is masked after exp with a 0/1 triangular multiply on the Pool
     engine (the depth-4 software pipeline hides the extra hop)
  4. ctx in [query, head-dim] orientation: the exp tile is the STATIONARY
     operand (stationary loads are free), so each (key-block, query-block,
     head) costs only 64+1 moving columns -- half the column count of the
     [head-dim, query] orientation. All 8 accumulators share one PSUM bank,
     so the bank is memset once per head pair and the matmuls accumulate
     with start=False (start_tensor_calc's 2KB zero region would clobber
     neighbours). Row sums accumulate as 1-column matmuls into a corner of
     a shared bank.
  5. normalize: per-partition reciprocal of the row sums ([128,8], one DVE
     op), one scalar-multiply per (query block, head), then a PE transpose
     (via identity, staged through bitcast bf16 slices of the shared PSUM
     bank) + VectorE copy into ctxt_sb's [e, token] layout
  6. out_partial = ctxT.T @ Wo_c.T    -> (2048, 1024) bf16 DMA'd out

Scheduling: all x chunk DMAs are issued up-front in priority order;
projection and output-projection work units are interleaved into the
exp-bound attention pipeline as PE filler; the first window's projections
run at half-chunk granularity so the PE starts as soon as the first DMAs
land; output-projection units for earlier windows are held back as filler
for the last window, whose drain units split their PSUM->SBUF copies
across ScalarE/VectorE and DMA each half-tile immediately.

Host: out[b] = sum of the 4 partials + (Wo @ bv + bo).
"""

import sys

for _p in ("/root/.axon_site/_ro/trn_rl_repo", "/opt/trn_rl_repo"):
    if _p not in sys.path:
        sys.path.append(_p)

from collections import deque
from functools import partial

import numpy as np
import ml_dtypes

import concourse.mybir as mybir
import concourse.tile as tile
from concourse import bacc, bass_utils

B, S, D, H = 2, 2048, 1024, 16
HD = D // H  # 64
NCORES = 8
EPC = 256  # embedding channels per core (4 heads)
BF16 = mybir.dt.bfloat16
F32 = mybir.dt.float32

_compiled_cache: dict[str, "bacc.Bacc"] = {}


def _kbs_for(qt: int, mode: str):
    """[(kb, q_lo)] for one 512-wide query tile."""
    if mode == "causal":
        return [(kb, 128 * (kb - 4 * qt) if kb >= 4 * qt else 0)
                for kb in range(4 * qt + 4)]
    return [(kb, 0) for kb in range(S // 128)]


def build_nc(mode: str = "causal") -> "bacc.Bacc":
    nc = bacc.Bacc("TRN2")

    xq = nc.dram_tensor("xq_t", (D, S), BF16, kind="ExternalInput")
    xk = nc.dram_tensor("xk_t", (D, S), BF16, kind="ExternalInput")
    xv = nc.dram_tensor("xv_t", (D, S), BF16, kind="ExternalInput")
    wq = nc.dram_tensor("wq_t", (D, EPC), BF16, kind="ExternalInput")
    wk = nc.dram_tensor("wk_t", (D, EPC), BF16, kind="ExternalInput")
    wv = nc.dram_tensor("wv_t", (D, EPC), BF16, kind="ExternalInput")
    wo = nc.dram_tensor("wo_t", (EPC, D), BF16, kind="ExternalInput")
    bqk = nc.dram_tensor("bqk2", (2, 2, 128, 1), F32, kind="ExternalInput")
    tri = nc.dram_tensor("tri", (128, 128), BF16, kind="ExternalInput")
    idn = nc.dram_tensor("idn", (128, 128), BF16, kind="ExternalInput")
    out = nc.dram_tensor("out", (S, D), BF16, kind="ExternalOutput")

    n_cb = D // 128  # 8 contraction blocks
    n_tb = S // 128  # 16 token blocks
    n_qt = S // 512  # 4 query tiles

    xq_v = xq.rearrange("(cb p) t -> p cb t", p=128)
    xk_v = xk.rearrange("(cb p) t -> p cb t", p=128)
    xv_v = xv.rearrange("(cb p) t -> p cb t", p=128)
    wq_v = wq.rearrange("(cb p) e -> p cb e", p=128)
    wk_v = wk.rearrange("(cb p) e -> p cb e", p=128)
    wv_v = wv.rearrange("(cb p) e -> p cb e", p=128)
    wo_v = wo.rearrange("(eb p) o -> p eb o", p=128)

    with tile.TileContext(nc) as tc:
        with (
            tc.tile_pool(name="consts", bufs=1) as consts,
            tc.tile_pool(name="qkt", bufs=1) as qkt_pool,
            tc.tile_pool(name="vaug", bufs=1) as vaug_pool,
            tc.tile_pool(name="ctxt", bufs=1) as ctxt_pool,
            tc.tile_pool(name="attn_sb", bufs=10) as attn_sb,
            tc.tile_pool(name="norm_sb", bufs=2) as norm_sb,
            tc.tile_pool(name="out_sb", bufs=8) as out_sb,
            # PSUM: 8 banks = psS 2x2 + psC 1x1 + psMM 2x1 + psRS 1x1
            tc.tile_pool(name="psS", bufs=2, space="PSUM") as psS,
            tc.tile_pool(name="psC", bufs=1, space="PSUM") as psC,
            tc.tile_pool(name="psMM", bufs=2, space="PSUM") as psMM,
            tc.tile_pool(name="psRS", bufs=1, space="PSUM") as psRS,
        ):
            # --- resident SBUF tensors ---
            xq_sb = consts.tile([128, n_cb * S], BF16, name="xq_sb")
            xk_sb = consts.tile([128, n_cb * S], BF16, name="xk_sb")
            xv_sb = consts.tile([128, n_cb * S], BF16, name="xv_sb")
            wq_sb = consts.tile([128, n_cb * EPC], BF16, name="wq_sb")
            wk_sb = consts.tile([128, n_cb * EPC], BF16, name="wk_sb")
            wv_sb = consts.tile([128, n_cb * EPC], BF16, name="wv_sb")
            wo_sb = consts.tile([128, 2 * D], BF16, name="wo_sb")
            tri_sb = consts.tile([128, 128], BF16, name="tri_sb")
            idn_sb = consts.tile([128, 128], BF16, name="idn_sb")
            # never written: garbage operand for PE warmup matmuls
            warm_sb = consts.tile([128, 256], BF16, name="warm_sb")
            bqk_sb = consts.tile([128, 4], F32, name="bqk_sb")
            bk_sb = bqk_sb[:, 0:2]
            bq_sb = bqk_sb[:, 2:4]

            qt_sb = [qkt_pool.tile([128, S], BF16, name=f"qt_sb{eb}", tag=f"qt{eb}")
                     for eb in range(2)]
            kt_sb = [qkt_pool.tile([128, S], BF16, name=f"kt_sb{eb}", tag=f"kt{eb}")
                     for eb in range(2)]
            vaug = [vaug_pool.tile([128, 4 * 65], BF16, name=f"vaug{tb}")
                    for tb in range(n_tb)]
            ctxt_sb = [ctxt_pool.tile([128, S], BF16, name=f"ctxt_sb{eb}")
                       for eb in range(2)]
            # one PSUM bank shared by the row-sum accumulators (cols 0-15)
            # and the two transpose staging slots (bitcast bf16, cols 64-191)
            rs_big = psRS.tile([128, 512], F32, name="rs_big")
            rs_ps = rs_big[:, 0:16].rearrange("p (a b) -> p a b", a=2)
            trp_slots = [
                rs_big[:, 64 + 64 * i:128 + 64 * i].bitcast(BF16)
                for i in range(2)
            ]

            def xs(t, cb):
                return t[:, cb * S:(cb + 1) * S]

            def ws(t, cb):
                return t[:, cb * EPC:(cb + 1) * EPC]

            def dma_one(xsb, dview, ts_):
                cs = slice(ts_ * 512, (ts_ + 1) * 512)
                nc.sync.dma_start(
                    xsb[:].rearrange("p (cb t) -> p cb t", cb=n_cb)[:, :, cs],
                    dview[:, :, cs],
                )

            def dma_half(xsb, dview, ts_, h):
                """Half-chunk (256-token) DMA: the first V blocks arrive
                ~1.6us earlier, unblocking the first ctx matmuls."""
                cs = slice(ts_ * 512 + h * 256, ts_ * 512 + (h + 1) * 256)
                nc.sync.dma_start(
                    xsb[:].rearrange("p (cb t) -> p cb t", cb=n_cb)[:, :, cs],
                    dview[:, :, cs],
                )

            def dma_chunk(ts_):
                """K and Q chunks now; the V chunk is deferred into the
                filler queue so it does not contend with the scores-critical
                K/Q DMAs at window start."""
                dma_one(xk_sb, xk_v, ts_)
                dma_one(xq_sb, xq_v, ts_)

            def qk_unit(xsb, wsb, bias, dst, eb, ts_):
                cs = slice(ts_ * 512, (ts_ + 1) * 512)
                ps = psMM.tile([128, 512], F32, name="proj_ps", tag="mm")
                for cb in range(n_cb):
                    nc.tensor.matmul(
                        ps[:],
                        lhsT=ws(wsb, cb)[:, eb * 128:(eb + 1) * 128],
                        rhs=xs(xsb, cb)[:, cs],
                        start=(cb == 0), stop=(cb == n_cb - 1),
                    )
                nc.vector.tensor_scalar_add(dst[eb][:, cs], ps[:],
                                            bias[:, eb:eb + 1])

            def qk_unit_half(xsb, wsb, bias, dst, eb, ts_, h):
                """256-token projection unit: paired with half-chunk DMAs at
                startup so the PE starts ~3us earlier."""
                cs = slice(ts_ * 512 + h * 256, ts_ * 512 + (h + 1) * 256)
                ps = psMM.tile([128, 512], F32, name="proj_ps", tag="mm")
                for cb in range(n_cb):
                    nc.tensor.matmul(
                        ps[:, 0:256],
                        lhsT=ws(wsb, cb)[:, eb * 128:(eb + 1) * 128],
                        rhs=xs(xsb, cb)[:, cs],
                        start=(cb == 0), stop=(cb == n_cb - 1),
                    )
                nc.vector.tensor_scalar_add(dst[eb][:, cs], ps[:, 0:256],
                                            bias[:, eb:eb + 1])

            def v_unit(tb):
                ps = psMM.tile([128, EPC], F32, name="v_ps", tag="mm")
                for cb in range(n_cb):
                    nc.tensor.matmul(
                        ps[:],
                        lhsT=xs(xv_sb, cb)[:, tb * 128:(tb + 1) * 128],
                        rhs=ws(wv_sb, cb),
                        start=(cb == 0), stop=(cb == n_cb - 1),
                    )
                va = vaug[tb].rearrange("p (h x) -> p h x", h=4)
                nc.vector.memset(va[:, :, 64:65], 1.0)
                nc.vector.tensor_copy(va[:, :, 0:64],
                                      ps.rearrange("p (h d) -> p h d", h=4))

            def proj_units(ts_):
                """Tagged units for chunk ts_.  K before Q for chunks whose K
                is needed early in their own window; for the last chunk Q
                leads because window 3 needs Q3 at step 0 while K3/V3 can be
                deferred into the window as filler (K3 read from step 12, V3
                from step 19)."""
                kq = [
                    (f"K{ts_}e{eb}",
                     partial(qk_unit, xk_sb, wk_sb, bk_sb, kt_sb, eb, ts_))
                    for eb in range(2)
                ]
                qq = [
                    (f"Q{ts_}e{eb}",
                     partial(qk_unit, xq_sb, wq_sb, bq_sb, qt_sb, eb, ts_))
                    for eb in range(2)
                ]
                units = qq + kq if ts_ == n_qt - 1 else kq + qq
                for tb in range(4 * ts_, 4 * ts_ + 4):
                    units.append((f"V{tb}", partial(v_unit, tb)))
                return units

            def outproj_unit(tb, tail=False):
                ot = out_sb.tile([128, D], BF16, name="out_t", tag="ot")
                if tail:
                    # drain path: alternate the PSUM->SBUF copies across
                    # ScalarE/VectorE and DMA each half as soon as it lands
                    for nb in range(2):
                        po = psMM.tile([128, 512], F32, name="out_ps",
                                       tag="mm")
                        for eb in range(2):
                            nc.tensor.matmul(
                                po[:],
                                lhsT=ctxt_sb[eb][:, tb * 128:(tb + 1) * 128],
                                rhs=wo_sb[:, eb * D + nb * 512:
                                          eb * D + (nb + 1) * 512],
                                start=(eb == 0), stop=(eb == 1),
                            )
                        if nb == 0:
                            nc.scalar.copy(ot[:, 0:512], po[:])
                        else:
                            nc.vector.tensor_copy(ot[:, 512:1024], po[:])
                        nc.sync.dma_start(
                            out[tb * 128:(tb + 1) * 128,
                                nb * 512:(nb + 1) * 512],
                            ot[:, nb * 512:(nb + 1) * 512])
                    return
                for nb in range(2):
                    po = psMM.tile([128, 512], F32, name="out_ps", tag="mm")
                    for eb in range(2):
                        nc.tensor.matmul(
                            po[:],
                            lhsT=ctxt_sb[eb][:, tb * 128:(tb + 1) * 128],
                            rhs=wo_sb[:, eb * D + nb * 512:
                                      eb * D + (nb + 1) * 512],
                            start=(eb == 0), stop=(eb == 1),
                        )
                    nc.vector.tensor_copy(ot[:, nb * 512:(nb + 1) * 512],
                                          po[:])
                nc.sync.dma_start(out[tb * 128:(tb + 1) * 128, :], ot[:])

            proj_q = deque()  # (tag, fn) projection units, deadline-scheduled
            emitted_tags = set()
            out_q = deque()  # holds token-block indices
            trans_q = deque()  # deferred ctxt transposes (highest priority)
            allow_out = [False]
            out_budget = [0]  # outproj pops allowed in the current window
            fill_debt = [0.0]  # ns of PE filler the exp pipeline is owed

            # estimated PE-engine cost of one popped filler unit (ns)
            def unit_cost(tag):
                return 1707.0 if tag[0] in "KQ" else 854.0

            def pop_ration(deficit_ns):
                """Deficit-based filler: accumulate (exp - PE) time per step
                and emit just enough filler to keep the PE fed.  Uniform
                1-pop-per-step both starves exp-heavy stretches (units are
                854-1707ns vs ~400ns/step deficit, so the reserve drains 2-4x
                too fast) and floods the DVE with PSUM->SBUF copies right
                before the norm chain needs it."""
                fill_debt[0] = max(fill_debt[0] + deficit_ns, -1300.0)
                while fill_debt[0] > 0:
                    if proj_q:
                        tag, fn = proj_q.popleft()
                        emitted_tags.add(tag)
                        fill_debt[0] -= unit_cost(tag)
                        fn()
                    elif out_q and allow_out[0] and out_budget[0] > 0:
                        out_budget[0] -= 1
                        fill_debt[0] -= 854.0
                        outproj_unit(out_q.popleft())
                    else:
                        break

            def force_tags(tags):
                """Emit any still-queued proj units bearing these tags NOW
                (data-dependency deadline), preserving queue order."""
                need = {t for t in tags if t not in emitted_tags}
                if not need:
                    return
                keep = deque()
                while proj_q:
                    tag, fn = proj_q.popleft()
                    if tag in need:
                        emitted_tags.add(tag)
                        fn()
                    else:
                        keep.append((tag, fn))
                proj_q.extend(keep)

            def attention(qt, last_window=False):
                # data-dependency deadlines (causal mode): window qt's scores
                # read K-chunk-qt only from step 4qt, and its ctx reads
                # V-chunk-qt only from step 4qt+LAG -- so those projection
                # units stay queued as filler with a forced-emission deadline
                # a couple of steps before first use.
                dl = {}
                if mode == "causal":
                    if qt >= 1:
                        dl[max(1, 4 * qt - 2)] = [f"K{qt}e{eb}"
                                                  for eb in range(2)]
                    dl[4 * qt + 5] = [f"V{tb}"
                                      for tb in range(4 * qt, 4 * qt + 4)]
                for hp in range(2):  # head pair (heads 2hp, 2hp+1)
                    kbs = _kbs_for(qt, mode)
                    # ctx accumulators in [query, head-dim] orientation: one
                    # PSUM bank holds all 4 query blocks x 2 heads x 64 dims;
                    # row sums accumulate as separate 1-column matmuls into
                    # the rs tile. The exp tile is the STATIONARY operand, so
                    # each (key-block, query-block, head) costs only 65
                    # moving columns instead of ~128.
                    pctx = psC.tile([128, 4, 2, 64], F32, name="pctx",
                                    tag="pc")
                    # multiple accumulators share these banks, so PSUM
                    # start_tensor_calc (2KB zero-region granularity) cannot
                    # be used: zero explicitly and accumulate with
                    # start=False throughout
                    nc.vector.memset(pctx[:], 0.0)
                    nc.vector.memset(rs_ps[:, hp, :], 0.0)
                    ets = {}
                    LAG = 9

                    def last_kb(qb):
                        return 4 * qt + qb if mode == "causal" else n_tb - 1

                    for i in range(len(kbs) + LAG):
                        drain_phase = i >= len(kbs)
                        if hp == 0 and i in dl:
                            force_tags(dl[i])
                        if i < len(kbs):
                            kb, q_lo = kbs[i]
                            w = 512 - q_lo
                            crossing = mode == "causal" and kb >= 4 * qt
                            ps = psS.tile([128, 1024], F32, name="sc_ps",
                                          tag="sc")
                            qs = qt * 512 + q_lo
                            for h2 in range(2):
                                nc.tensor.matmul(
                                    ps[:, 512 * h2 + q_lo:512 * h2 + 512],
                                    lhsT=kt_sb[hp][64 * h2:64 * h2 + 64,
                                                   kb * 128:(kb + 1) * 128],
                                    rhs=qt_sb[hp][64 * h2:64 * h2 + 64,
                                                  qs:qs + w],
                                )
                            et = attn_sb.tile([128, 1024], BF16, name="exp_t",
                                              tag="exp")
                            psg = ps.rearrange("p (g c) -> p g c", g=2)
                            etg = et.rearrange("p (g c) -> p g c", g=2)
                            nc.scalar.activation(
                                etg[:, :, q_lo:512], psg[:, :, q_lo:512],
                                mybir.ActivationFunctionType.Exp,
                                scale=0.125,
                            )
                            if crossing:
                                # zero the masked upper half of the diagonal
                                # block with a 0/1 multiply on the (otherwise
                                # idle) Pool engine; the LAG-deep pipeline
                                # hides the extra hop
                                dg = etg[:, :, q_lo:q_lo + 128]
                                nc.gpsimd.tensor_mul(
                                    dg, dg,
                                    tri_sb[:, None, :].broadcast_to(
                                        [128, 2, 128]),
                                )
                            ets[i] = et
                        if i >= LAG:
                            kb, q_lo = kbs[i - LAG]
                            et = ets.pop(i - LAG)
                            etg = et.rearrange("p (g c) -> p g c", g=2)
                            qb_lo = max(0, kb - 4 * qt) \
                                if mode == "causal" else 0
                            for h2 in range(2):
                                hh = 2 * hp + h2
                                for qb in range(qb_lo, 4):
                                    c0 = 128 * qb  # et cols are absolute
                                    stat = etg[:, h2, c0:c0 + 128]
                                    nc.tensor.matmul(
                                        pctx[:, qb, h2, :],
                                        lhsT=stat,
                                        rhs=vaug[kb][:, 65 * hh:65 * hh + 64],
                                        start=False,
                                        stop=(kb == last_kb(qb)),
                                        skip_group_check=True,
                                    )
                                    nc.tensor.matmul(
                                        rs_ps[:, hp, 2 * qb + h2:
                                              2 * qb + h2 + 1],
                                        lhsT=stat,
                                        rhs=vaug[kb][:, 65 * hh + 64:
                                                     65 * hh + 65],
                                        start=False,
                                        stop=(kb == last_kb(qb)),
                                        skip_group_check=True,
                                    )
                        # deferred transposes from step 4 on (one per step):
                        # late enough that the previous hp's DVE multiply
                        # chain has drained, early enough (< LAG) that all 4
                        # are emitted before this hp's first row-sum write
                        if i >= 4 and trans_q:
                            trans_q.popleft()()
                        # deficit-rationed filler
                        deficit = 0.0
                        if i < len(kbs):
                            deficit += (2 * w * 0.8333 + 242.0) \
                                - 2 * w * 0.4167
                        if i >= LAG:
                            kb_c, _ = kbs[i - LAG]
                            qbl = max(0, kb_c - 4 * qt) \
                                if mode == "causal" else 0
                            deficit -= 2 * (4 - qbl) * 65 * 0.4167
                        pop_ration(deficit)
                    # normalize: per-partition reciprocal of the row sums,
                    # then one scalar-multiply per (query block, head) into
                    # the [q, e] staging tile, and DMA-transpose each
                    # 128x128 block into ctxt_sb's [e, token] layout
                    # flush deferred transposes before reusing a cq buffer:
                    # the pool only orders against EMITTED readers, so a
                    # still-queued transpose would read clobbered data
                    while trans_q:
                        trans_q.popleft()()
                    rec = norm_sb.tile([128, 8], F32, name="rec", tag="rec")
                    with nc.allow_low_precision(reason="softmax 1/rowsum"):
                        nc.vector.reciprocal(rec[:], rs_ps[:, hp, :])
                    cq = norm_sb.tile([128, 4, 128], BF16, name="cq",
                                      tag="cq")
                    # ONE broadcast tensor_tensor multiply instead of 8
                    # tensor_scalar ops: the reciprocal [128,4,2] broadcasts
                    # along the 64 head dims, cutting the norm chain from
                    # ~2.3us (8 serial DVE ops) to ~0.9us and freeing DVE
                    # throughput for the copies the transposes depend on
                    recv = rec[:].rearrange("p (a b) -> p a b", a=4) \
                        [:, :, :, None].broadcast_to([128, 4, 2, 64])
                    with nc.allow_low_precision(reason="softmax normalize"):
                        nc.vector.tensor_mul(
                            cq[:].rearrange("p a (b d) -> p a b d", b=2),
                            pctx[:], recv)
                    for qb in range(4):
                        # PE transpose back to [e, token] orientation
                        # (stationary load is free; 128 moving columns), then
                        # a VectorE copy into ctxt_sb
                        # defer the transpose: emitted inline it would sit in
                        # the in-order PE stream waiting on the DVE multiply,
                        # delaying the next window's score matmuls behind it.
                        # As a high-priority filler it runs a few steps into
                        # the next window, when the multiply has long drained.
                        def do_transpose(hp=hp, qt=qt, qb=qb, cq=cq):
                            trp = trp_slots[qb % 2]
                            nc.tensor.transpose(trp, cq[:, qb, :], idn_sb[:])
                            nc.vector.tensor_copy(
                                ctxt_sb[hp][:, qt * 512 + 128 * qb:
                                            qt * 512 + 128 * qb + 128],
                                trp,
                            )
                        trans_q.append(do_transpose)
                    # refill the pipeline bubble: PE work emitted after the
                    # norm chain runs while the next head pair's exp warms up.
                    # The transposes just queued above must NOT pop here --
                    # they wait for the DVE multiply chain and would stall the
                    # in-order PE stream; they pop at the next hp's steps 3+.
                    pop_ration(800.0)

            # --- emission ---
            def dma_xpiece(xsb, dview, cb_lo, cb_hi, t_lo, t_hi):
                nc.sync.dma_start(
                    xsb[:].rearrange("p (cb t) -> p cb t", cb=n_cb)
                    [:, cb_lo:cb_hi, t_lo:t_hi],
                    dview[:, cb_lo:cb_hi, t_lo:t_hi],
                )

            def dma_wpiece(wsb, wview, cb_lo, cb_hi):
                nc.sync.dma_start(
                    wsb[:].rearrange("p (cb e) -> p cb e", cb=n_cb)
                    [:, cb_lo:cb_hi],
                    wview[:, cb_lo:cb_hi],
                )

            # PE warmup: the tensor engine runs at half speed until it has
            # been continuously busy for 3us, and the startup is DMA-paced
            # (the PE consumes each arriving piece faster than the next one
            # lands).  Matmuls on a dummy SBUF tile (results land in the psS
            # rotation and are fully overwritten by the first start=True
            # scores matmuls) keep the ramp going; a few are interleaved
            # between the first DMA-gated projection units to bridge the
            # arrival gaps.
            nc.vector.memset(warm_sb[:], 0.0)

            def warm(n):
                for _ in range(n):
                    wps = psS.tile([128, 1024], F32, name="warm_ps",
                                   tag="sc")
                    nc.tensor.matmul(
                        wps[:, 0:256],
                        lhsT=warm_sb[:, 0:128],
                        rhs=warm_sb[:],
                        start=True, stop=True,
                    )

            warm(11)

            # startup: the first projection matmul needs only wk[cb0-3] +
            # xk[cb0-3, first 256 tokens], so split those transfers in half;
            # the PE starts ~2us earlier than with monolithic DMAs.  bias /
            # tri / idn constants ride early: the first bias-add needs bk_sb
            # (a late bias DMA stalls the psMM slot rotation), and tri gates
            # the Pool mask-multiply on window 0's diagonal blocks.
            dma_wpiece(wk_sb, wk_v, 0, 4)
            dma_xpiece(xk_sb, xk_v, 0, 4, 0, 256)
            dma_wpiece(wk_sb, wk_v, 4, 8)
            dma_xpiece(xk_sb, xk_v, 4, 8, 0, 256)
            nc.sync.dma_start(
                bqk_sb[:].rearrange("p (s eb) -> p s eb", s=2),
                bqk.rearrange("s eb p x -> p s (eb x)"))
            nc.sync.dma_start(tri_sb[:], tri[:])
            dma_xpiece(xk_sb, xk_v, 0, n_cb, 256, 512)
            dma_wpiece(wq_sb, wq_v, 0, 4)
            dma_xpiece(xq_sb, xq_v, 0, 4, 0, 256)
            dma_wpiece(wq_sb, wq_v, 4, 8)
            dma_xpiece(xq_sb, xq_v, 4, 8, 0, 256)
            dma_xpiece(xq_sb, xq_v, 0, n_cb, 256, 512)
            nc.sync.dma_start(idn_sb[:], idn[:])
            nc.sync.dma_start(
                wv_sb[:].rearrange("p (cb e) -> p cb e", cb=n_cb), wv_v[:])
            dma_half(xv_sb, xv_v, 0, 0)
            dma_half(xv_sb, xv_v, 0, 1)
            # wo before the bulk chunks: outproj units become legal filler
            # from window 1 on
            nc.sync.dma_start(
                wo_sb[:].rearrange("p (eb o) -> p eb o", eb=2), wo_v[:])
            # all remaining x chunks up-front in priority order: the DMA
            # engine drains this FIFO while the PE works, so later windows'
            # projections never stall on data supply.  chunk 1 sends Q first
            # (window 1 needs Q1 at step 0, K1 only from step 4).
            for g in range(1, n_qt):
                if g == 1:
                    dma_one(xq_sb, xq_v, g)
                    dma_one(xk_sb, xk_v, g)
                else:
                    dma_chunk(g)
                dma_half(xv_sb, xv_v, g, 0)
                dma_half(xv_sb, xv_v, g, 1)

            # group 0 K/Q inline at half-chunk granularity (DMA-paced);
            # interleaved warmups bridge the piece-arrival gaps
            winter = iter((3, 2, 1, 1, 1, 0, 0, 0))
            for xsb, wsb, bias, dst in (
                (xk_sb, wk_sb, bk_sb, kt_sb),
                (xq_sb, wq_sb, bq_sb, qt_sb),
            ):
                for h in range(2):
                    for eb in range(2):
                        qk_unit_half(xsb, wsb, bias, dst, eb, 0, h)
                        warm(next(winter))
            proj_q.extend((f"V{tb}", partial(v_unit, tb)) for tb in range(4))
            qt_order = [0, 1, 2, 3]

            def window_req(qt):
                """Tags that must be emitted before attention(qt) starts."""
                req = []
                hi = qt if mode == "causal" else n_qt
                for g in range(1, hi):  # chunks 1..qt-1 fully
                    req += [f"K{g}e0", f"K{g}e1", f"Q{g}e0", f"Q{g}e1"]
                    req += [f"V{t}" for t in range(4 * g, 4 * g + 4)]
                if mode == "causal" and qt >= 1:
                    req += [f"Q{qt}e0", f"Q{qt}e1"]
                return req

            for wi, qt in enumerate(qt_order):
                if mode == "causal":
                    if wi + 1 < n_qt:
                        units = proj_units(wi + 1)
                        if wi + 1 == n_qt - 1:
                            # queue only Q3 now; hold K3/V3 back as dedicated
                            # window-3 filler (that window has no projection
                            # work of its own and the most exp-bound steps)
                            proj_q.extend(u for u in units
                                          if u[0][0] == "Q")
                            defer_units = [u for u in units
                                           if u[0][0] != "Q"]
                        else:
                            proj_q.extend(units)
                elif wi == 0:  # full mask: window 0 consumes every chunk
                    for g in range(1, n_qt):
                        proj_q.extend(proj_units(g))
                if mode == "causal" and wi == n_qt - 1:
                    proj_q.extend(defer_units)
                force_tags(window_req(qt))
                allow_out[0] = wi >= 1
                out_budget[0] = (0, 1, 3, 999)[wi]
                attention(qt, last_window=(wi == 3))
                out_q.extend(range(4 * qt, 4 * qt + 4))
            while trans_q:  # last window's deferred ctxt transposes
                trans_q.popleft()()
            while proj_q:  # full-mask mode can leave units queued
                tag, fn = proj_q.popleft()
                fn()
            while out_q:
                # tail units split their PSUM->SBUF copies across engines
                outproj_unit(out_q.popleft(), tail=True)

    nc.compile()
    return nc


def get_compiled(mode: str = "causal") -> "bacc.Bacc":
    nc = _compiled_cache.get(mode)
    if nc is None:
        nc = build_nc(mode)
        _compiled_cache[mode] = nc
    return nc


def kernel(query, key, value, mask, Wq, bq, Wk, bk, Wv, bv, Wo, bo):
    query = np.asarray(query, np.float32)
    key = np.asarray(key, np.float32)
    value = np.asarray(value, np.float32)
    mask = np.asarray(mask)
    Wq, bq = np.asarray(Wq, np.float32), np.asarray(bq, np.float32)
    Wk, bk = np.asarray(Wk, np.float32), np.asarray(bk, np.float32)
    Wv, bv = np.asarray(Wv, np.float32), np.asarray(bv, np.float32)
    Wo, bo = np.asarray(Wo, np.float32), np.asarray(bo, np.float32)

    trilm = np.tril(np.ones((S, S), mask.dtype))
    if all(np.array_equal(mask[b], trilm) for b in range(B)):
        mode = "causal"
    elif mask.all():
        mode = "full"
    else:
        raise NotImplementedError("general mask not supported")

    bf = ml_dtypes.bfloat16
    xT = {}
    for nm, arr in (("q", query), ("k", key), ("v", value)):
        xT[nm] = [np.ascontiguousarray(arr[b].T).astype(bf) for b in range(B)]
    WqT = Wq.T.astype(bf)
    WkT = Wk.T.astype(bf)
    WvT = Wv.T.astype(bf)
    WoT = np.ascontiguousarray(Wo.T).astype(bf)
    tri_np = np.where(
        np.arange(128)[:, None] <= np.arange(128)[None, :], 1.0, 0.0
    ).astype(bf)

    in_maps = []
    for c in range(NCORES):
        b, hb = c // 4, c % 4
        es = hb * EPC
        in_maps.append({
            "xq_t": xT["q"][b],
            "xk_t": xT["k"][b],
            "xv_t": xT["v"][b],
            "wq_t": np.ascontiguousarray(WqT[:, es:es + EPC]),
            "wk_t": np.ascontiguousarray(WkT[:, es:es + EPC]),
            "wv_t": np.ascontiguousarray(WvT[:, es:es + EPC]),
            "wo_t": np.ascontiguousarray(WoT[es:es + EPC, :]),
            "bqk2": np.stack([
                bk[es:es + EPC].reshape(2, 128, 1),
                bq[es:es + EPC].reshape(2, 128, 1),
            ]).astype(np.float32),
            "tri": tri_np,
            "idn": np.eye(128, dtype=np.float32).astype(bf),
        })

    nc = get_compiled(mode)
    res = bass_utils.run_bass_kernel_spmd(nc, in_maps, core_ids=list(range(NCORES)))

    const = Wo @ bv + bo
    outf = np.zeros((B, S, D), np.float32)
    for c in range(NCORES):
        outf[c // 4] += res.results[c]["out"].astype(np.float32)
    outf += const[None, None, :]
    return outf



# revision 30
# speedup vs baseline: 1.0061x; 1.0061x over previous
"""Trainium2 Bass kernel for 16-head causal MultiHeadAttention.

Problem: B=2, S=2048, D=1024, H=16 (head_dim 64), causal mask, f32 I/O.

Sharding (8 cores): core c handles batch b = c//4 and head-block hb = c%4
(4 heads = 256 embedding channels). Q/K/V projections are tensor-parallel
column slices; the output projection is tensor-parallel over rows of Wo.T,
so each core emits a partial (S, D) output that the host sums per batch.

Per-core kernel (all matmuls bf16, f32 accumulate):
  1. QT/KT = (Wq_c.T).T @ xT (+bias)   -> (256, 2048) SBUF, e in partitions
  2. V    = xT.T @ Wv_c.T              -> (2048, 256) "V_aug" layout with a
     ones column per head (for softmax row sums)
  3. scoresT[k, q] = KT.T @ QT per 128-key block (both heads of a pair per
     pass), causal blocks only; exp on ScalarE (scale=1/8, no max
     subtraction -- scores are O(5) so exp is safe in f32); the diagonal
     block is masked after exp with a 0/1 triangular multiply on the Pool
     engine (the depth-4 software pipeline hides the extra hop)
  4. ctx in [query, head-dim] orientation: the exp tile is the STATIONARY
     operand (stationary loads are free), so each (key-block, query-block,
     head) costs only 64+1 moving columns -- half the column count of the
     [head-dim, query] orientation. All 8 accumulators share one PSUM bank,
     so the bank is memset once per head pair and the matmuls accumulate
     with start=False (start_tensor_calc's 2KB zero region would clobber
     neighbours). Row sums accumulate as 1-column matmuls into a corner of
     a shared bank.
  5. normalize: per-partition reciprocal of the row sums ([128,8], one DVE
     op), one scalar-multiply per (query block, head), then a PE transpose
     (via identity, staged through bitcast bf16 slices of the shared PSUM
     bank) + VectorE copy into ctxt_sb's [e, token] layout
  6. out_partial = ctxT.T @ Wo_c.T    -> (2048, 1024) bf16 DMA'd out

Scheduling: all x chunk DMAs are issued up-front in priority order;
projection and output-projection work units are interleaved into the
exp-bound attention pipeline as PE filler; the first window's projections
run at half-chunk granularity so the PE starts as soon as the first DMAs
land; output-projection units for earlier windows are held back as filler
for the last window, whose drain units split their PSUM->SBUF copies
across ScalarE/VectorE and DMA each half-tile immediately.

Host: out[b] = sum of the 4 partials + (Wo @ bv + bo).
"""

import sys

for _p in ("/root/.axon_site/_ro/trn_rl_repo", "/opt/trn_rl_repo"):
    if _p not in sys.path:
        sys.path.append(_p)

from collections import deque
from functools import partial

import numpy as np
import ml_dtypes

import concourse.mybir as mybir
import concourse.tile as tile
from concourse import bacc, bass_utils

B, S, D, H = 2, 2048, 1024, 16
HD = D // H  # 64
NCORES = 8
EPC = 256  # embedding channels per core (4 heads)
BF16 = mybir.dt.bfloat16
F32 = mybir.dt.float32

_compiled_cache: dict[str, "bacc.Bacc"] = {}


def _kbs_for(qt: int, mode: str):
    """[(kb, q_lo)] for one 512-wide query tile."""
    if mode == "causal":
        return [(kb, 128 * (kb - 4 * qt) if kb >= 4 * qt else 0)
                for kb in range(4 * qt + 4)]
    return [(kb, 0) for kb in range(S // 128)]


def build_nc(mode: str = "causal") -> "bacc.Bacc":
    nc = bacc.Bacc("TRN2")

    xq = nc.dram_tensor("xq_t", (D, S), BF16, kind="ExternalInput")
    xk = nc.dram_tensor("xk_t", (D, S), BF16, kind="ExternalInput")
    xv = nc.dram_tensor("xv_t", (D, S), BF16, kind="ExternalInput")
    wq = nc.dram_tensor("wq_t", (D, EPC), BF16, kind="ExternalInput")
    wk = nc.dram_tensor("wk_t", (D, EPC), BF16, kind="ExternalInput")
    wv = nc.dram_tensor("wv_t", (D, EPC), BF16, kind="ExternalInput")
    wo = nc.dram_tensor("wo_t", (EPC, D), BF16, kind="ExternalInput")
    bqk = nc.dram_tensor("bqk2", (2, 2, 128, 1), F32, kind="ExternalInput")
    tri = nc.dram_tensor("tri", (128, 128), BF16, kind="ExternalInput")
    idn = nc.dram_tensor("idn", (128, 128), BF16, kind="ExternalInput")
    out = nc.dram_tensor("out", (S, D), BF16, kind="ExternalOutput")

    n_cb = D // 128  # 8 contraction blocks
    n_tb = S // 128  # 16 token blocks
    n_qt = S // 512  # 4 query tiles

    xq_v = xq.rearrange("(cb p) t -> p cb t", p=128)
    xk_v = xk.rearrange("(cb p) t -> p cb t", p=128)
    xv_v = xv.rearrange("(cb p) t -> p cb t", p=128)
    wq_v = wq.rearrange("(cb p) e -> p cb e", p=128)
    wk_v = wk.rearrange("(cb p) e -> p cb e", p=128)
    wv_v = wv.rearrange("(cb p) e -> p cb e", p=128)
    wo_v = wo.rearrange("(eb p) o -> p eb o", p=128)

    with tile.TileContext(nc) as tc:
        with (
            tc.tile_pool(name="consts", bufs=1) as consts,
            tc.tile_pool(name="qkt", bufs=1) as qkt_pool,
            tc.tile_pool(name="vaug", bufs=1) as vaug_pool,
            tc.tile_pool(name="ctxt", bufs=1) as ctxt_pool,
            tc.tile_pool(name="attn_sb", bufs=10) as attn_sb,
            tc.tile_pool(name="norm_sb", bufs=2) as norm_sb,
            tc.tile_pool(name="out_sb", bufs=8) as out_sb,
            # PSUM: 8 banks = psS 2x2 + psC 1x1 + psMM 2x1 + psRS 1x1
            tc.tile_pool(name="psS", bufs=2, space="PSUM") as psS,
            tc.tile_pool(name="psC", bufs=1, space="PSUM") as psC,
            tc.tile_pool(name="psMM", bufs=2, space="PSUM") as psMM,
            tc.tile_pool(name="psRS", bufs=1, space="PSUM") as psRS,
        ):
            # --- resident SBUF tensors ---
            xq_sb = consts.tile([128, n_cb * S], BF16, name="xq_sb")
            xk_sb = consts.tile([128, n_cb * S], BF16, name="xk_sb")
            xv_sb = consts.tile([128, n_cb * S], BF16, name="xv_sb")
            wq_sb = consts.tile([128, n_cb * EPC], BF16, name="wq_sb")
            wk_sb = consts.tile([128, n_cb * EPC], BF16, name="wk_sb")
            wv_sb = consts.tile([128, n_cb * EPC], BF16, name="wv_sb")
            wo_sb = consts.tile([128, 2 * D], BF16, name="wo_sb")
            tri_sb = consts.tile([128, 128], BF16, name="tri_sb")
            idn_sb = consts.tile([128, 128], BF16, name="idn_sb")
            # never written: garbage operand for PE warmup matmuls
            warm_sb = consts.tile([128, 256], BF16, name="warm_sb")
            bqk_sb = consts.tile([128, 4], F32, name="bqk_sb")
            bk_sb = bqk_sb[:, 0:2]
            bq_sb = bqk_sb[:, 2:4]

            qt_sb = [qkt_pool.tile([128, S], BF16, name=f"qt_sb{eb}", tag=f"qt{eb}")
                     for eb in range(2)]
            kt_sb = [qkt_pool.tile([128, S], BF16, name=f"kt_sb{eb}", tag=f"kt{eb}")
                     for eb in range(2)]
            vaug = [vaug_pool.tile([128, 4 * 65], BF16, name=f"vaug{tb}")
                    for tb in range(n_tb)]
            ctxt_sb = [ctxt_pool.tile([128, S], BF16, name=f"ctxt_sb{eb}")
                       for eb in range(2)]
            # one PSUM bank shared by the row-sum accumulators (cols 0-15)
            # and FOUR transpose staging slots (bitcast bf16, cols 64-319).
            # Four slots let all 4 of a head pair's transposes run
            # back-to-back on the in-order PE (with 2 slots, transpose qb+2
            # waits for qb's DVE copy -- a ~620ns PE->DVE->PE ping-pong per
            # block that also stalls every instruction queued behind it)
            rs_big = psRS.tile([128, 512], F32, name="rs_big")
            rs_ps = rs_big[:, 0:16].rearrange("p (a b) -> p a b", a=2)
            trp_slots = [
                rs_big[:, 64 + 64 * i:128 + 64 * i].bitcast(BF16)
                for i in range(4)
            ]

            def xs(t, cb):
                return t[:, cb * S:(cb + 1) * S]

            def ws(t, cb):
                return t[:, cb * EPC:(cb + 1) * EPC]

            def dma_one(xsb, dview, ts_):
                cs = slice(ts_ * 512, (ts_ + 1) * 512)
                nc.sync.dma_start(
                    xsb[:].rearrange("p (cb t) -> p cb t", cb=n_cb)[:, :, cs],
                    dview[:, :, cs],
                )

            def dma_half(xsb, dview, ts_, h):
                """Half-chunk (256-token) DMA: the first V blocks arrive
                ~1.6us earlier, unblocking the first ctx matmuls."""
                cs = slice(ts_ * 512 + h * 256, ts_ * 512 + (h + 1) * 256)
                nc.sync.dma_start(
                    xsb[:].rearrange("p (cb t) -> p cb t", cb=n_cb)[:, :, cs],
                    dview[:, :, cs],
                )

            def dma_chunk(ts_):
                """K and Q chunks now; the V chunk is deferred into the
                filler queue so it does not contend with the scores-critical
                K/Q DMAs at window start."""
                dma_one(xk_sb, xk_v, ts_)
                dma_one(xq_sb, xq_v, ts_)

            def qk_unit(xsb, wsb, bias, dst, eb, ts_):
                cs = slice(ts_ * 512, (ts_ + 1) * 512)
                ps = psMM.tile([128, 512], F32, name="proj_ps", tag="mm")
                for cb in range(n_cb):
                    nc.tensor.matmul(
                        ps[:],
                        lhsT=ws(wsb, cb)[:, eb * 128:(eb + 1) * 128],
                        rhs=xs(xsb, cb)[:, cs],
                        start=(cb == 0), stop=(cb == n_cb - 1),
                    )
                nc.vector.tensor_scalar_add(dst[eb][:, cs], ps[:],
                                            bias[:, eb:eb + 1])

            def qk_unit_half(xsb, wsb, bias, dst, eb, ts_, h):
                """256-token projection unit: paired with half-chunk DMAs at
                startup so the PE starts ~3us earlier."""
                cs = slice(ts_ * 512 + h * 256, ts_ * 512 + (h + 1) * 256)
                ps = psMM.tile([128, 512], F32, name="proj_ps", tag="mm")
                for cb in range(n_cb):
                    nc.tensor.matmul(
                        ps[:, 0:256],
                        lhsT=ws(wsb, cb)[:, eb * 128:(eb + 1) * 128],
                        rhs=xs(xsb, cb)[:, cs],
                        start=(cb == 0), stop=(cb == n_cb - 1),
                    )
                nc.vector.tensor_scalar_add(dst[eb][:, cs], ps[:, 0:256],
                                            bias[:, eb:eb + 1])

            def v_unit(tb):
                ps = psMM.tile([128, EPC], F32, name="v_ps", tag="mm")
                for cb in range(n_cb):
                    nc.tensor.matmul(
                        ps[:],
                        lhsT=xs(xv_sb, cb)[:, tb * 128:(tb + 1) * 128],
                        rhs=ws(wv_sb, cb),
                        start=(cb == 0), stop=(cb == n_cb - 1),
                    )
                va = vaug[tb].rearrange("p (h x) -> p h x", h=4)
                nc.vector.memset(va[:, :, 64:65], 1.0)
                nc.vector.tensor_copy(va[:, :, 0:64],
                                      ps.rearrange("p (h d) -> p h d", h=4))

            def proj_units(ts_):
                """Tagged units for chunk ts_.  K before Q for chunks whose K
                is needed early in their own window; for the last chunk Q
                leads because window 3 needs Q3 at step 0 while K3/V3 can be
                deferred into the window as filler (K3 read from step 12, V3
                from step 19)."""
                kq = [
                    (f"K{ts_}e{eb}",
                     partial(qk_unit, xk_sb, wk_sb, bk_sb, kt_sb, eb, ts_))
                    for eb in range(2)
                ]
                qq = [
                    (f"Q{ts_}e{eb}",
                     partial(qk_unit, xq_sb, wq_sb, bq_sb, qt_sb, eb, ts_))
                    for eb in range(2)
                ]
                units = qq + kq if ts_ == n_qt - 1 else kq + qq
                for tb in range(4 * ts_, 4 * ts_ + 4):
                    units.append((f"V{tb}", partial(v_unit, tb)))
                return units

            def outproj_unit(tb, tail=False):
                ot = out_sb.tile([128, D], BF16, name="out_t", tag="ot")
                if tail:
                    # drain path: alternate the PSUM->SBUF copies across
                    # ScalarE/VectorE and DMA each half as soon as it lands
                    for nb in range(2):
                        po = psMM.tile([128, 512], F32, name="out_ps",
                                       tag="mm")
                        for eb in range(2):
                            nc.tensor.matmul(
                                po[:],
                                lhsT=ctxt_sb[eb][:, tb * 128:(tb + 1) * 128],
                                rhs=wo_sb[:, eb * D + nb * 512:
                                          eb * D + (nb + 1) * 512],
                                start=(eb == 0), stop=(eb == 1),
                            )
                        if nb == 0:
                            nc.scalar.copy(ot[:, 0:512], po[:])
                        else:
                            nc.vector.tensor_copy(ot[:, 512:1024], po[:])
                        nc.sync.dma_start(
                            out[tb * 128:(tb + 1) * 128,
                                nb * 512:(nb + 1) * 512],
                            ot[:, nb * 512:(nb + 1) * 512])
                    return
                for nb in range(2):
                    po = psMM.tile([128, 512], F32, name="out_ps", tag="mm")
                    for eb in range(2):
                        nc.tensor.matmul(
                            po[:],
                            lhsT=ctxt_sb[eb][:, tb * 128:(tb + 1) * 128],
                            rhs=wo_sb[:, eb * D + nb * 512:
                                      eb * D + (nb + 1) * 512],
                            start=(eb == 0), stop=(eb == 1),
                        )
                    nc.vector.tensor_copy(ot[:, nb * 512:(nb + 1) * 512],
                                          po[:])
                nc.sync.dma_start(out[tb * 128:(tb + 1) * 128, :], ot[:])

            proj_q = deque()  # (tag, fn) projection units, deadline-scheduled
            emitted_tags = set()
            out_q = deque()  # holds token-block indices
            trans_q = deque()  # deferred ctxt transposes (highest priority)
            allow_out = [False]
            out_budget = [0]  # outproj pops allowed in the current window
            fill_debt = [0.0]  # ns of PE filler the exp pipeline is owed

            # estimated PE-engine cost of one popped filler unit (ns)
            def unit_cost(tag):
                return 1707.0 if tag[0] in "KQ" else 854.0

            def pop_ration(deficit_ns):
                """Deficit-based filler: accumulate (exp - PE) time per step
                and emit just enough filler to keep the PE fed.  Uniform
                1-pop-per-step both starves exp-heavy stretches (units are
                854-1707ns vs ~400ns/step deficit, so the reserve drains 2-4x
                too fast) and floods the DVE with PSUM->SBUF copies right
                before the norm chain needs it."""
                fill_debt[0] = max(fill_debt[0] + deficit_ns, -1300.0)
                while fill_debt[0] > 0:
                    if proj_q:
                        tag, fn = proj_q.popleft()
                        emitted_tags.add(tag)
                        fill_debt[0] -= unit_cost(tag)
                        fn()
                    elif out_q and allow_out[0] and out_budget[0] > 0:
                        out_budget[0] -= 1
                        fill_debt[0] -= 854.0
                        outproj_unit(out_q.popleft())
                    else:
                        break

            def force_tags(tags):
                """Emit any still-queued proj units bearing these tags NOW
                (data-dependency deadline), preserving queue order."""
                need = {t for t in tags if t not in emitted_tags}
                if not need:
                    return
                keep = deque()
                while proj_q:
                    tag, fn = proj_q.popleft()
                    if tag in need:
                        emitted_tags.add(tag)
                        fn()
                    else:
                        keep.append((tag, fn))
                proj_q.extend(keep)

            def attention(qt, last_window=False):
                # data-dependency deadlines (causal mode): window qt's scores
                # read K-chunk-qt only from step 4qt, and its ctx reads
                # V-chunk-qt only from step 4qt+LAG -- so those projection
                # units stay queued as filler with a forced-emission deadline
                # a couple of steps before first use.
                dl = {}
                if mode == "causal":
                    if qt >= 1:
                        dl[max(1, 4 * qt - 2)] = [f"K{qt}e{eb}"
                                                  for eb in range(2)]
                    dl[4 * qt + 5] = [f"V{tb}"
                                      for tb in range(4 * qt, 4 * qt + 4)]
                for hp in range(2):  # head pair (heads 2hp, 2hp+1)
                    kbs = _kbs_for(qt, mode)
                    # ctx accumulators in [query, head-dim] orientation: one
                    # PSUM bank holds all 4 query blocks x 2 heads x 64 dims;
                    # row sums accumulate as separate 1-column matmuls into
                    # the rs tile. The exp tile is the STATIONARY operand, so
                    # each (key-block, query-block, head) costs only 65
                    # moving columns instead of ~128.
                    pctx = psC.tile([128, 4, 2, 64], F32, name="pctx",
                                    tag="pc")
                    # multiple accumulators share these banks, so PSUM
                    # start_tensor_calc (2KB zero-region granularity) cannot
                    # be used: zero explicitly and accumulate with
                    # start=False throughout
                    nc.vector.memset(pctx[:], 0.0)
                    nc.vector.memset(rs_ps[:, hp, :], 0.0)
                    ets = {}
                    LAG = 9

                    def last_kb(qb):
                        return 4 * qt + qb if mode == "causal" else n_tb - 1

                    for i in range(len(kbs) + LAG):
                        drain_phase = i >= len(kbs)
                        if hp == 0 and i in dl:
                            force_tags(dl[i])
                        if i < len(kbs):
                            kb, q_lo = kbs[i]
                            w = 512 - q_lo
                            crossing = mode == "causal" and kb >= 4 * qt
                            ps = psS.tile([128, 1024], F32, name="sc_ps",
                                          tag="sc")
                            qs = qt * 512 + q_lo
                            for h2 in range(2):
                                nc.tensor.matmul(
                                    ps[:, 512 * h2 + q_lo:512 * h2 + 512],
                                    lhsT=kt_sb[hp][64 * h2:64 * h2 + 64,
                                                   kb * 128:(kb + 1) * 128],
                                    rhs=qt_sb[hp][64 * h2:64 * h2 + 64,
                                                  qs:qs + w],
                                )
                            et = attn_sb.tile([128, 1024], BF16, name="exp_t",
                                              tag="exp")
                            psg = ps.rearrange("p (g c) -> p g c", g=2)
                            etg = et.rearrange("p (g c) -> p g c", g=2)
                            nc.scalar.activation(
                                etg[:, :, q_lo:512], psg[:, :, q_lo:512],
                                mybir.ActivationFunctionType.Exp,
                                scale=0.125,
                            )
                            if crossing:
                                # zero the masked upper half of the diagonal
                                # block with a 0/1 multiply on the (otherwise
                                # idle) Pool engine; the LAG-deep pipeline
                                # hides the extra hop
                                dg = etg[:, :, q_lo:q_lo + 128]
                                nc.gpsimd.tensor_mul(
                                    dg, dg,
                                    tri_sb[:, None, :].broadcast_to(
                                        [128, 2, 128]),
                                )
                            ets[i] = et
                        if i >= LAG:
                            kb, q_lo = kbs[i - LAG]
                            et = ets.pop(i - LAG)
                            etg = et.rearrange("p (g c) -> p g c", g=2)
                            qb_lo = max(0, kb - 4 * qt) \
                                if mode == "causal" else 0
                            for h2 in range(2):
                                hh = 2 * hp + h2
                                for qb in range(qb_lo, 4):
                                    c0 = 128 * qb  # et cols are absolute
                                    stat = etg[:, h2, c0:c0 + 128]
                                    nc.tensor.matmul(
                                        pctx[:, qb, h2, :],
                                        lhsT=stat,
                                        rhs=vaug[kb][:, 65 * hh:65 * hh + 64],
                                        start=False,
                                        stop=(kb == last_kb(qb)),
                                        skip_group_check=True,
                                    )
                                    nc.tensor.matmul(
                                        rs_ps[:, hp, 2 * qb + h2:
                                              2 * qb + h2 + 1],
                                        lhsT=stat,
                                        rhs=vaug[kb][:, 65 * hh + 64:
                                                     65 * hh + 65],
                                        start=False,
                                        stop=(kb == last_kb(qb)),
                                        skip_group_check=True,
                                    )
                        # deferred transposes from step 4 on (one per step):
                        # late enough that the previous hp's DVE multiply
                        # chain has drained, early enough (< LAG) that all 4
                        # are emitted before this hp's first row-sum write
                        if i >= 4 and trans_q:
                            trans_q.popleft()()
                        # deficit-rationed filler
                        deficit = 0.0
                        if i < len(kbs):
                            deficit += (2 * w * 0.8333 + 200.0) \
                                - 2 * w * 0.4167
                        if i >= LAG:
                            kb_c, _ = kbs[i - LAG]
                            qbl = max(0, kb_c - 4 * qt) \
                                if mode == "causal" else 0
                            deficit -= 2 * (4 - qbl) * 65 * 0.4167
                        pop_ration(deficit)
                    # normalize: per-partition reciprocal of the row sums,
                    # then one scalar-multiply per (query block, head) into
                    # the [q, e] staging tile, and DMA-transpose each
                    # 128x128 block into ctxt_sb's [e, token] layout
                    # flush deferred transposes before reusing a cq buffer:
                    # the pool only orders against EMITTED readers, so a
                    # still-queued transpose would read clobbered data
                    while trans_q:
                        trans_q.popleft()()
                    rec = norm_sb.tile([128, 8], F32, name="rec", tag="rec")
                    with nc.allow_low_precision(reason="softmax 1/rowsum"):
                        nc.vector.reciprocal(rec[:], rs_ps[:, hp, :])
                    cq = norm_sb.tile([128, 4, 128], BF16, name="cq",
                                      tag="cq")
                    # ONE broadcast tensor_tensor multiply instead of 8
                    # tensor_scalar ops: the reciprocal [128,4,2] broadcasts
                    # along the 64 head dims, cutting the norm chain from
                    # ~2.3us (8 serial DVE ops) to ~0.9us and freeing DVE
                    # throughput for the copies the transposes depend on
                    recv = rec[:].rearrange("p (a b) -> p a b", a=4) \
                        [:, :, :, None].broadcast_to([128, 4, 2, 64])
                    with nc.allow_low_precision(reason="softmax normalize"):
                        nc.vector.tensor_mul(
                            cq[:].rearrange("p a (b d) -> p a b d", b=2),
                            pctx[:], recv)
                    for qb in range(4):
                        # PE transpose back to [e, token] orientation
                        # (stationary load is free; 128 moving columns), then
                        # a VectorE copy into ctxt_sb
                        # defer the transpose: emitted inline it would sit in
                        # the in-order PE stream waiting on the DVE multiply,
                        # delaying the next window's score matmuls behind it.
                        # As a high-priority filler it runs a few steps into
                        # the next window, when the multiply has long drained.
                        def do_transpose(hp=hp, qt=qt, qb=qb, cq=cq):
                            trp = trp_slots[qb]
                            nc.tensor.transpose(trp, cq[:, qb, :], idn_sb[:])
                            nc.vector.tensor_copy(
                                ctxt_sb[hp][:, qt * 512 + 128 * qb:
                                            qt * 512 + 128 * qb + 128],
                                trp,
                            )
                        trans_q.append(do_transpose)
                    # refill the pipeline bubble: PE work emitted after the
                    # norm chain runs while the next head pair's exp warms up.
                    # The transposes just queued above must NOT pop here --
                    # they wait for the DVE multiply chain and would stall the
                    # in-order PE stream; they pop at the next hp's steps 3+.
                    pop_ration(800.0)

            # --- emission ---
            def dma_xpiece(xsb, dview, cb_lo, cb_hi, t_lo, t_hi):
                nc.sync.dma_start(
                    xsb[:].rearrange("p (cb t) -> p cb t", cb=n_cb)
                    [:, cb_lo:cb_hi, t_lo:t_hi],
                    dview[:, cb_lo:cb_hi, t_lo:t_hi],
                )

            def dma_wpiece(wsb, wview, cb_lo, cb_hi):
                nc.sync.dma_start(
                    wsb[:].rearrange("p (cb e) -> p cb e", cb=n_cb)
                    [:, cb_lo:cb_hi],
                    wview[:, cb_lo:cb_hi],
                )

            # PE warmup: the tensor engine runs at half speed until it has
            # been continuously busy for 3us, and the startup is DMA-paced
            # (the PE consumes each arriving piece faster than the next one
            # lands).  Matmuls on a dummy SBUF tile (results land in the psS
            # rotation and are fully overwritten by the first start=True
            # scores matmuls) keep the ramp going; a few are interleaved
            # between the first DMA-gated projection units to bridge the
            # arrival gaps.
            nc.vector.memset(warm_sb[:], 0.0)

            def warm(n):
                for _ in range(n):
                    wps = psS.tile([128, 1024], F32, name="warm_ps",
                                   tag="sc")
                    nc.tensor.matmul(
                        wps[:, 0:256],
                        lhsT=warm_sb[:, 0:128],
                        rhs=warm_sb[:],
                        start=True, stop=True,
                    )

            warm(11)

            # startup: the first projection matmul needs only wk[cb0-3] +
            # xk[cb0-3, first 256 tokens], so split those transfers in half;
            # the PE starts ~2us earlier than with monolithic DMAs.  bias /
            # tri / idn constants ride early: the first bias-add needs bk_sb
            # (a late bias DMA stalls the psMM slot rotation), and tri gates
            # the Pool mask-multiply on window 0's diagonal blocks.
            dma_wpiece(wk_sb, wk_v, 0, 4)
            dma_xpiece(xk_sb, xk_v, 0, 4, 0, 256)
            dma_wpiece(wk_sb, wk_v, 4, 8)
            dma_xpiece(xk_sb, xk_v, 4, 8, 0, 256)
            nc.sync.dma_start(
                bqk_sb[:].rearrange("p (s eb) -> p s eb", s=2),
                bqk.rearrange("s eb p x -> p s (eb x)"))
            nc.sync.dma_start(tri_sb[:], tri[:])
            dma_xpiece(xk_sb, xk_v, 0, n_cb, 256, 512)
            dma_wpiece(wq_sb, wq_v, 0, 4)
            dma_xpiece(xq_sb, xq_v, 0, 4, 0, 256)
            dma_wpiece(wq_sb, wq_v, 4, 8)
            dma_xpiece(xq_sb, xq_v, 4, 8, 0, 256)
            dma_xpiece(xq_sb, xq_v, 0, n_cb, 256, 512)
            nc.sync.dma_start(idn_sb[:], idn[:])
            nc.sync.dma_start(
                wv_sb[:].rearrange("p (cb e) -> p cb e", cb=n_cb), wv_v[:])
            dma_half(xv_sb, xv_v, 0, 0)
            dma_half(xv_sb, xv_v, 0, 1)
            # wo before the bulk chunks: outproj units become legal filler
            # from window 1 on
            nc.sync.dma_start(
                wo_sb[:].rearrange("p (eb o) -> p eb o", eb=2), wo_v[:])
            # all remaining x chunks up-front in priority order: the DMA
            # engine drains this FIFO while the PE works, so later windows'
            # projections never stall on data supply.  chunk 1 sends Q first
            # (window 1 needs Q1 at step 0, K1 only from step 4).
            for g in range(1, n_qt):
                if g == 1:
                    dma_one(xq_sb, xq_v, g)
                    dma_one(xk_sb, xk_v, g)
                else:
                    dma_chunk(g)
                dma_half(xv_sb, xv_v, g, 0)
                dma_half(xv_sb, xv_v, g, 1)

            # group 0 K/Q inline at half-chunk granularity (DMA-paced);
            # interleaved warmups bridge the piece-arrival gaps
            winter = iter((3, 2, 1, 1, 1, 0, 0, 0))
            for xsb, wsb, bias, dst in (
                (xk_sb, wk_sb, bk_sb, kt_sb),
                (xq_sb, wq_sb, bq_sb, qt_sb),
            ):
                for h in range(2):
                    for eb in range(2):
                        qk_unit_half(xsb, wsb, bias, dst, eb, 0, h)
                        warm(next(winter))
            proj_q.extend((f"V{tb}", partial(v_unit, tb)) for tb in range(4))
            qt_order = [0, 1, 2, 3]

            def window_req(qt):
                """Tags that must be emitted before attention(qt) starts."""
                req = []
                hi = qt if mode == "causal" else n_qt
                for g in range(1, hi):  # chunks 1..qt-1 fully
                    req += [f"K{g}e0", f"K{g}e1", f"Q{g}e0", f"Q{g}e1"]
                    req += [f"V{t}" for t in range(4 * g, 4 * g + 4)]
                if mode == "causal" and qt >= 1:
                    req += [f"Q{qt}e0", f"Q{qt}e1"]
                return req

            for wi, qt in enumerate(qt_order):
                if mode == "causal":
                    if wi + 1 < n_qt:
                        units = proj_units(wi + 1)
                        if wi + 1 == n_qt - 1:
                            # queue only Q3 now; hold K3/V3 back as dedicated
                            # window-3 filler (that window has no projection
                            # work of its own and the most exp-bound steps)
                            proj_q.extend(u for u in units
                                          if u[0][0] == "Q")
                            defer_units = [u for u in units
                                           if u[0][0] != "Q"]
                        else:
                            proj_q.extend(units)
                elif wi == 0:  # full mask: window 0 consumes every chunk
                    for g in range(1, n_qt):
                        proj_q.extend(proj_units(g))
                if mode == "causal" and wi == n_qt - 1:
                    proj_q.extend(defer_units)
                force_tags(window_req(qt))
                allow_out[0] = wi >= 1
                out_budget[0] = (0, 1, 3, 999)[wi]
                attention(qt, last_window=(wi == 3))
                out_q.extend(range(4 * qt, 4 * qt + 4))
            while trans_q:  # last window's deferred ctxt transposes
                trans_q.popleft()()
            while proj_q:  # full-mask mode can leave units queued
                tag, fn = proj_q.popleft()
                fn()
            while out_q:
                # tail units split their PSUM->SBUF copies across engines
                outproj_unit(out_q.popleft(), tail=True)

    nc.compile()
    return nc


def get_compiled(mode: str = "causal") -> "bacc.Bacc":
    nc = _compiled_cache.get(mode)
    if nc is None:
        nc = build_nc(mode)
        _compiled_cache[mode] = nc
    return nc


def kernel(query, key, value, mask, Wq, bq, Wk, bk, Wv, bv, Wo, bo):
    query = np.asarray(query, np.float32)
    key = np.asarray(key, np.float32)
    value = np.asarray(value, np.float32)
    mask = np.asarray(mask)
    Wq, bq = np.asarray(Wq, np.float32), np.asarray(bq, np.float32)
    Wk, bk = np.asarray(Wk, np.float32), np.asarray(bk, np.float32)
    Wv, bv = np.asarray(Wv, np.float32), np.asarray(bv, np.float32)
    Wo, bo = np.asarray(Wo, np.float32), np.asarray(bo, np.float32)

    trilm = np.tril(np.ones((S, S), mask.dtype))
    if all(np.array_equal(mask[b], trilm) for b in range(B)):
        mode = "causal"
    elif mask.all():
        mode = "full"
    else:
        raise NotImplementedError("general mask not supported")

    bf = ml_dtypes.bfloat16
    xT = {}
    for nm, arr in (("q", query), ("k", key), ("v", value)):
        xT[nm] = [np.ascontiguousarray(arr[b].T).astype(bf) for b in range(B)]
    WqT = Wq.T.astype(bf)
    WkT = Wk.T.astype(bf)
    WvT = Wv.T.astype(bf)
    WoT = np.ascontiguousarray(Wo.T).astype(bf)
    tri_np = np.where(
        np.arange(128)[:, None] <= np.arange(128)[None, :], 1.0, 0.0
    ).astype(bf)

    in_maps = []
    for c in range(NCORES):
        b, hb = c // 4, c % 4
        es = hb * EPC
        in_maps.append({
            "xq_t": xT["q"][b],
            "xk_t": xT["k"][b],
            "xv_t": xT["v"][b],
            "wq_t": np.ascontiguousarray(WqT[:, es:es + EPC]),
            "wk_t": np.ascontiguousarray(WkT[:, es:es + EPC]),
            "wv_t": np.ascontiguousarray(WvT[:, es:es + EPC]),
            "wo_t": np.ascontiguousarray(WoT[es:es + EPC, :]),
            "bqk2": np.stack([
                bk[es:es + EPC].reshape(2, 128, 1),
                bq[es:es + EPC].reshape(2, 128, 1),
            ]).astype(np.float32),
            "tri": tri_np,
            "idn": np.eye(128, dtype=np.float32).astype(bf),
        })

    nc = get_compiled(mode)
    res = bass_utils.run_bass_kernel_spmd(nc, in_maps, core_ids=list(range(NCORES)))

    const = Wo @ bv + bo
    outf = np.zeros((B, S, D), np.float32)
    for c in range(NCORES):
        outf[c // 4] += res.results[c]["out"].astype(np.float32)
    outf += const[None, None, :]
    return outf



# revision 31
# speedup vs baseline: 1.0067x; 1.0006x over previous
"""Trainium2 Bass kernel for 16-head causal MultiHeadAttention.

Problem: B=2, S=2048, D=1024, H=16 (head_dim 64), causal mask, f32 I/O.

Sharding (8 cores): core c handles batch b = c//4 and head-block hb = c%4
(4 heads = 256 embedding channels). Q/K/V projections are tensor-parallel
column slices; the output projection is tensor-parallel over rows of Wo.T,
so each core emits a partial (S, D) output that the host sums per batch.

Per-core kernel (all matmuls bf16, f32 accumulate):
  1. QT/KT = (Wq_c.T).T @ xT (+bias)   -> (256, 2048) SBUF, e in partitions
  2. V    = xT.T @ Wv_c.T              -> (2048, 256) "V_aug" layout with a
     ones column per head (for softmax row sums)
  3. scoresT[k, q] = KT.T @ QT per 128-key block (both heads of a pair per
     pass), causal blocks only; exp on ScalarE (scale=1/8, no max
     subtraction -- scores are O(5) so exp is safe in f32); the diagonal
     block is masked after exp with a 0/1 triangular multiply on the Pool
     engine (the depth-4 software pipeline hides the extra hop)
  4. ctx in [query, head-dim] orientation: the exp tile is the STATIONARY
     operand (stationary loads are free), so each (key-block, query-block,
     head) costs only 64+1 moving columns -- half the column count of the
     [head-dim, query] orientation. All 8 accumulators share one PSUM bank,
     so the bank is memset once per head pair and the matmuls accumulate
     with start=False (start_tensor_calc's 2KB zero region would clobber
     neighbours). Row sums accumulate as 1-column matmuls into a corner of
     a shared bank.
  5. normalize: per-partition reciprocal of the row sums ([128,8], one DVE
     op), one scalar-multiply per (query block, head), then a PE transpose
     (via identity, staged through bitcast bf16 slices of the shared PSUM
     bank) + VectorE copy into ctxt_sb's [e, token] layout
  6. out_partial = ctxT.T @ Wo_c.T    -> (2048, 1024) bf16 DMA'd out

Scheduling: all x chunk DMAs are issued up-front in priority order;
projection and output-projection work units are interleaved into the
exp-bound attention pipeline as PE filler; the first window's projections
run at half-chunk granularity so the PE starts as soon as the first DMAs
land; output-projection units for earlier windows are held back as filler
for the last window, whose drain units split their PSUM->SBUF copies
across ScalarE/VectorE and DMA each half-tile immediately.

Host: out[b] = sum of the 4 partials + (Wo @ bv + bo).
"""

import sys

for _p in ("/root/.axon_site/_ro/trn_rl_repo", "/opt/trn_rl_repo"):
    if _p not in sys.path:
        sys.path.append(_p)

from collections import deque
from functools import partial

import numpy as np
import ml_dtypes

import concourse.mybir as mybir
import concourse.tile as tile
from concourse import bacc, bass_utils

B, S, D, H = 2, 2048, 1024, 16
HD = D // H  # 64
NCORES = 8
EPC = 256  # embedding channels per core (4 heads)
BF16 = mybir.dt.bfloat16
F32 = mybir.dt.float32

_compiled_cache: dict[str, "bacc.Bacc"] = {}


def _kbs_for(qt: int, mode: str):
    """[(kb, q_lo)] for one 512-wide query tile."""
    if mode == "causal":
        return [(kb, 128 * (kb - 4 * qt) if kb >= 4 * qt else 0)
                for kb in range(4 * qt + 4)]
    return [(kb, 0) for kb in range(S // 128)]


def build_nc(mode: str = "causal") -> "bacc.Bacc":
    nc = bacc.Bacc("TRN2")

    xq = nc.dram_tensor("xq_t", (D, S), BF16, kind="ExternalInput")
    xk = nc.dram_tensor("xk_t", (D, S), BF16, kind="ExternalInput")
    xv = nc.dram_tensor("xv_t", (D, S), BF16, kind="ExternalInput")
    wq = nc.dram_tensor("wq_t", (D, EPC), BF16, kind="ExternalInput")
    wk = nc.dram_tensor("wk_t", (D, EPC), BF16, kind="ExternalInput")
    wv = nc.dram_tensor("wv_t", (D, EPC), BF16, kind="ExternalInput")
    wo = nc.dram_tensor("wo_t", (EPC, D), BF16, kind="ExternalInput")
    bqk = nc.dram_tensor("bqk2", (2, 2, 128, 1), F32, kind="ExternalInput")
    tri = nc.dram_tensor("tri", (128, 128), BF16, kind="ExternalInput")
    idn = nc.dram_tensor("idn", (128, 128), BF16, kind="ExternalInput")
    out = nc.dram_tensor("out", (S, D), BF16, kind="ExternalOutput")

    n_cb = D // 128  # 8 contraction blocks
    n_tb = S // 128  # 16 token blocks
    n_qt = S // 512  # 4 query tiles

    xq_v = xq.rearrange("(cb p) t -> p cb t", p=128)
    xk_v = xk.rearrange("(cb p) t -> p cb t", p=128)
    xv_v = xv.rearrange("(cb p) t -> p cb t", p=128)
    wq_v = wq.rearrange("(cb p) e -> p cb e", p=128)
    wk_v = wk.rearrange("(cb p) e -> p cb e", p=128)
    wv_v = wv.rearrange("(cb p) e -> p cb e", p=128)
    wo_v = wo.rearrange("(eb p) o -> p eb o", p=128)

    with tile.TileContext(nc) as tc:
        with (
            tc.tile_pool(name="consts", bufs=1) as consts,
            tc.tile_pool(name="qkt", bufs=1) as qkt_pool,
            tc.tile_pool(name="vaug", bufs=1) as vaug_pool,
            tc.tile_pool(name="ctxt", bufs=1) as ctxt_pool,
            tc.tile_pool(name="attn_sb", bufs=10) as attn_sb,
            tc.tile_pool(name="norm_sb", bufs=2) as norm_sb,
            tc.tile_pool(name="out_sb", bufs=8) as out_sb,
            # PSUM: 8 banks = psS 2x2 + psC 1x1 + psMM 2x1 + psRS 1x1
            tc.tile_pool(name="psS", bufs=2, space="PSUM") as psS,
            tc.tile_pool(name="psC", bufs=1, space="PSUM") as psC,
            tc.tile_pool(name="psMM", bufs=2, space="PSUM") as psMM,
            tc.tile_pool(name="psRS", bufs=1, space="PSUM") as psRS,
        ):
            # --- resident SBUF tensors ---
            xq_sb = consts.tile([128, n_cb * S], BF16, name="xq_sb")
            xk_sb = consts.tile([128, n_cb * S], BF16, name="xk_sb")
            xv_sb = consts.tile([128, n_cb * S], BF16, name="xv_sb")
            wq_sb = consts.tile([128, n_cb * EPC], BF16, name="wq_sb")
            wk_sb = consts.tile([128, n_cb * EPC], BF16, name="wk_sb")
            wv_sb = consts.tile([128, n_cb * EPC], BF16, name="wv_sb")
            wo_sb = consts.tile([128, 2 * D], BF16, name="wo_sb")
            tri_sb = consts.tile([128, 128], BF16, name="tri_sb")
            idn_sb = consts.tile([128, 128], BF16, name="idn_sb")
            # never written: garbage operand for PE warmup matmuls
            warm_sb = consts.tile([128, 256], BF16, name="warm_sb")
            bqk_sb = consts.tile([128, 4], F32, name="bqk_sb")
            bk_sb = bqk_sb[:, 0:2]
            bq_sb = bqk_sb[:, 2:4]

            qt_sb = [qkt_pool.tile([128, S], BF16, name=f"qt_sb{eb}", tag=f"qt{eb}")
                     for eb in range(2)]
            kt_sb = [qkt_pool.tile([128, S], BF16, name=f"kt_sb{eb}", tag=f"kt{eb}")
                     for eb in range(2)]
            vaug = [vaug_pool.tile([128, 4 * 65], BF16, name=f"vaug{tb}")
                    for tb in range(n_tb)]
            ctxt_sb = [ctxt_pool.tile([128, S], BF16, name=f"ctxt_sb{eb}")
                       for eb in range(2)]
            # one PSUM bank shared by the row-sum accumulators (cols 0-15)
            # and FOUR transpose staging slots (bitcast bf16, cols 64-319).
            # Four slots let all 4 of a head pair's transposes run
            # back-to-back on the in-order PE (with 2 slots, transpose qb+2
            # waits for qb's DVE copy -- a ~620ns PE->DVE->PE ping-pong per
            # block that also stalls every instruction queued behind it)
            rs_big = psRS.tile([128, 512], F32, name="rs_big")
            rs_ps = rs_big[:, 0:16].rearrange("p (a b) -> p a b", a=2)
            trp_slots = [
                rs_big[:, 64 + 64 * i:128 + 64 * i].bitcast(BF16)
                for i in range(4)
            ]

            def xs(t, cb):
                return t[:, cb * S:(cb + 1) * S]

            def ws(t, cb):
                return t[:, cb * EPC:(cb + 1) * EPC]

            def dma_one(xsb, dview, ts_):
                cs = slice(ts_ * 512, (ts_ + 1) * 512)
                nc.sync.dma_start(
                    xsb[:].rearrange("p (cb t) -> p cb t", cb=n_cb)[:, :, cs],
                    dview[:, :, cs],
                )

            def dma_half(xsb, dview, ts_, h):
                """Half-chunk (256-token) DMA: the first V blocks arrive
                ~1.6us earlier, unblocking the first ctx matmuls."""
                cs = slice(ts_ * 512 + h * 256, ts_ * 512 + (h + 1) * 256)
                nc.sync.dma_start(
                    xsb[:].rearrange("p (cb t) -> p cb t", cb=n_cb)[:, :, cs],
                    dview[:, :, cs],
                )

            def dma_chunk(ts_):
                """K and Q chunks now; the V chunk is deferred into the
                filler queue so it does not contend with the scores-critical
                K/Q DMAs at window start."""
                dma_one(xk_sb, xk_v, ts_)
                dma_one(xq_sb, xq_v, ts_)

            def qk_unit(xsb, wsb, bias, dst, eb, ts_):
                cs = slice(ts_ * 512, (ts_ + 1) * 512)
                ps = psMM.tile([128, 512], F32, name="proj_ps", tag="mm")
                for cb in range(n_cb):
                    nc.tensor.matmul(
                        ps[:],
                        lhsT=ws(wsb, cb)[:, eb * 128:(eb + 1) * 128],
                        rhs=xs(xsb, cb)[:, cs],
                        start=(cb == 0), stop=(cb == n_cb - 1),
                    )
                nc.vector.tensor_scalar_add(dst[eb][:, cs], ps[:],
                                            bias[:, eb:eb + 1])

            def qk_unit_half(xsb, wsb, bias, dst, eb, ts_, h):
                """256-token projection unit: paired with half-chunk DMAs at
                startup so the PE starts ~3us earlier."""
                cs = slice(ts_ * 512 + h * 256, ts_ * 512 + (h + 1) * 256)
                ps = psMM.tile([128, 512], F32, name="proj_ps", tag="mm")
                for cb in range(n_cb):
                    nc.tensor.matmul(
                        ps[:, 0:256],
                        lhsT=ws(wsb, cb)[:, eb * 128:(eb + 1) * 128],
                        rhs=xs(xsb, cb)[:, cs],
                        start=(cb == 0), stop=(cb == n_cb - 1),
                    )
                nc.vector.tensor_scalar_add(dst[eb][:, cs], ps[:, 0:256],
                                            bias[:, eb:eb + 1])

            def v_unit(tb):
                ps = psMM.tile([128, EPC], F32, name="v_ps", tag="mm")
                for cb in range(n_cb):
                    nc.tensor.matmul(
                        ps[:],
                        lhsT=xs(xv_sb, cb)[:, tb * 128:(tb + 1) * 128],
                        rhs=ws(wv_sb, cb),
                        start=(cb == 0), stop=(cb == n_cb - 1),
                    )
                va = vaug[tb].rearrange("p (h x) -> p h x", h=4)
                nc.vector.memset(va[:, :, 64:65], 1.0)
                nc.vector.tensor_copy(va[:, :, 0:64],
                                      ps.rearrange("p (h d) -> p h d", h=4))

            def proj_units(ts_):
                """Tagged units for chunk ts_.  K before Q for chunks whose K
                is needed early in their own window; for the last chunk Q
                leads because window 3 needs Q3 at step 0 while K3/V3 can be
                deferred into the window as filler (K3 read from step 12, V3
                from step 19)."""
                kq = [
                    (f"K{ts_}e{eb}",
                     partial(qk_unit, xk_sb, wk_sb, bk_sb, kt_sb, eb, ts_))
                    for eb in range(2)
                ]
                qq = [
                    (f"Q{ts_}e{eb}",
                     partial(qk_unit, xq_sb, wq_sb, bq_sb, qt_sb, eb, ts_))
                    for eb in range(2)
                ]
                units = qq + kq if ts_ == n_qt - 1 else kq + qq
                for tb in range(4 * ts_, 4 * ts_ + 4):
                    units.append((f"V{tb}", partial(v_unit, tb)))
                return units

            def outproj_unit(tb, tail=False):
                ot = out_sb.tile([128, D], BF16, name="out_t", tag="ot")
                if tail:
                    # drain path: alternate the PSUM->SBUF copies across
                    # ScalarE/VectorE and DMA each half as soon as it lands
                    for nb in range(2):
                        po = psMM.tile([128, 512], F32, name="out_ps",
                                       tag="mm")
                        for eb in range(2):
                            nc.tensor.matmul(
                                po[:],
                                lhsT=ctxt_sb[eb][:, tb * 128:(tb + 1) * 128],
                                rhs=wo_sb[:, eb * D + nb * 512:
                                          eb * D + (nb + 1) * 512],
                                start=(eb == 0), stop=(eb == 1),
                            )
                        if nb == 0:
                            nc.scalar.copy(ot[:, 0:512], po[:])
                        else:
                            nc.vector.tensor_copy(ot[:, 512:1024], po[:])
                        nc.sync.dma_start(
                            out[tb * 128:(tb + 1) * 128,
                                nb * 512:(nb + 1) * 512],
                            ot[:, nb * 512:(nb + 1) * 512])
                    return
                for nb in range(2):
                    po = psMM.tile([128, 512], F32, name="out_ps", tag="mm")
                    for eb in range(2):
                        nc.tensor.matmul(
                            po[:],
                            lhsT=ctxt_sb[eb][:, tb * 128:(tb + 1) * 128],
                            rhs=wo_sb[:, eb * D + nb * 512:
                                      eb * D + (nb + 1) * 512],
                            start=(eb == 0), stop=(eb == 1),
                        )
                    nc.vector.tensor_copy(ot[:, nb * 512:(nb + 1) * 512],
                                          po[:])
                nc.sync.dma_start(out[tb * 128:(tb + 1) * 128, :], ot[:])

            proj_q = deque()  # (tag, fn) projection units, deadline-scheduled
            emitted_tags = set()
            out_q = deque()  # holds token-block indices
            trans_q = deque()  # deferred ctxt transposes (highest priority)
            allow_out = [False]
            out_budget = [0]  # outproj pops allowed in the current window
            fill_debt = [0.0]  # ns of PE filler the exp pipeline is owed

            # estimated PE-engine cost of one popped filler unit (ns)
            def unit_cost(tag):
                return 1707.0 if tag[0] in "KQ" else 854.0

            def pop_ration(deficit_ns):
                """Deficit-based filler: accumulate (exp - PE) time per step
                and emit just enough filler to keep the PE fed.  Uniform
                1-pop-per-step both starves exp-heavy stretches (units are
                854-1707ns vs ~400ns/step deficit, so the reserve drains 2-4x
                too fast) and floods the DVE with PSUM->SBUF copies right
                before the norm chain needs it."""
                fill_debt[0] = max(fill_debt[0] + deficit_ns, -1300.0)
                while fill_debt[0] > 0:
                    if proj_q:
                        tag, fn = proj_q.popleft()
                        emitted_tags.add(tag)
                        fill_debt[0] -= unit_cost(tag)
                        fn()
                    elif out_q and allow_out[0] and out_budget[0] > 0:
                        out_budget[0] -= 1
                        fill_debt[0] -= 854.0
                        outproj_unit(out_q.popleft())
                    else:
                        break

            def force_tags(tags):
                """Emit any still-queued proj units bearing these tags NOW
                (data-dependency deadline), preserving queue order."""
                need = {t for t in tags if t not in emitted_tags}
                if not need:
                    return
                keep = deque()
                while proj_q:
                    tag, fn = proj_q.popleft()
                    if tag in need:
                        emitted_tags.add(tag)
                        fn()
                    else:
                        keep.append((tag, fn))
                proj_q.extend(keep)

            def attention(qt, last_window=False):
                # data-dependency deadlines (causal mode): window qt's scores
                # read K-chunk-qt only from step 4qt, and its ctx reads
                # V-chunk-qt only from step 4qt+LAG -- so those projection
                # units stay queued as filler with a forced-emission deadline
                # a couple of steps before first use.
                dl = {}
                if mode == "causal":
                    if qt >= 1:
                        dl[max(1, 4 * qt - 2)] = [f"K{qt}e{eb}"
                                                  for eb in range(2)]
                    dl[4 * qt + 5] = [f"V{tb}"
                                      for tb in range(4 * qt, 4 * qt + 4)]
                for hp in range(2):  # head pair (heads 2hp, 2hp+1)
                    kbs = _kbs_for(qt, mode)
                    # ctx accumulators in [query, head-dim] orientation: one
                    # PSUM bank holds all 4 query blocks x 2 heads x 64 dims;
                    # row sums accumulate as separate 1-column matmuls into
                    # the rs tile. The exp tile is the STATIONARY operand, so
                    # each (key-block, query-block, head) costs only 65
                    # moving columns instead of ~128.
                    pctx = psC.tile([128, 4, 2, 64], F32, name="pctx",
                                    tag="pc")
                    # multiple accumulators share these banks, so PSUM
                    # start_tensor_calc (2KB zero-region granularity) cannot
                    # be used: zero explicitly and accumulate with
                    # start=False throughout
                    nc.vector.memset(pctx[:], 0.0)
                    nc.vector.memset(rs_ps[:, hp, :], 0.0)
                    ets = {}
                    LAG = 9

                    def last_kb(qb):
                        return 4 * qt + qb if mode == "causal" else n_tb - 1

                    for i in range(len(kbs) + LAG):
                        drain_phase = i >= len(kbs)
                        if hp == 0 and i in dl:
                            force_tags(dl[i])
                        if i < len(kbs):
                            kb, q_lo = kbs[i]
                            w = 512 - q_lo
                            crossing = mode == "causal" and kb >= 4 * qt
                            ps = psS.tile([128, 1024], F32, name="sc_ps",
                                          tag="sc")
                            qs = qt * 512 + q_lo
                            for h2 in range(2):
                                nc.tensor.matmul(
                                    ps[:, 512 * h2 + q_lo:512 * h2 + 512],
                                    lhsT=kt_sb[hp][64 * h2:64 * h2 + 64,
                                                   kb * 128:(kb + 1) * 128],
                                    rhs=qt_sb[hp][64 * h2:64 * h2 + 64,
                                                  qs:qs + w],
                                )
                            et = attn_sb.tile([128, 1024], BF16, name="exp_t",
                                              tag="exp")
                            psg = ps.rearrange("p (g c) -> p g c", g=2)
                            etg = et.rearrange("p (g c) -> p g c", g=2)
                            nc.scalar.activation(
                                etg[:, :, q_lo:512], psg[:, :, q_lo:512],
                                mybir.ActivationFunctionType.Exp,
                                scale=0.125,
                            )
                            if crossing:
                                # zero the masked upper half of the diagonal
                                # block with a 0/1 multiply on the (otherwise
                                # idle) Pool engine; the LAG-deep pipeline
                                # hides the extra hop
                                dg = etg[:, :, q_lo:q_lo + 128]
                                nc.gpsimd.tensor_mul(
                                    dg, dg,
                                    tri_sb[:, None, :].broadcast_to(
                                        [128, 2, 128]),
                                )
                            ets[i] = et
                        if i >= LAG:
                            kb, q_lo = kbs[i - LAG]
                            et = ets.pop(i - LAG)
                            etg = et.rearrange("p (g c) -> p g c", g=2)
                            qb_lo = max(0, kb - 4 * qt) \
                                if mode == "causal" else 0
                            for h2 in range(2):
                                hh = 2 * hp + h2
                                for qb in range(qb_lo, 4):
                                    c0 = 128 * qb  # et cols are absolute
                                    stat = etg[:, h2, c0:c0 + 128]
                                    nc.tensor.matmul(
                                        pctx[:, qb, h2, :],
                                        lhsT=stat,
                                        rhs=vaug[kb][:, 65 * hh:65 * hh + 64],
                                        start=False,
                                        stop=(kb == last_kb(qb)),
                                        skip_group_check=True,
                                    )
                                    nc.tensor.matmul(
                                        rs_ps[:, hp, 2 * qb + h2:
                                              2 * qb + h2 + 1],
                                        lhsT=stat,
                                        rhs=vaug[kb][:, 65 * hh + 64:
                                                     65 * hh + 65],
                                        start=False,
                                        stop=(kb == last_kb(qb)),
                                        skip_group_check=True,
                                    )
                        # deferred transposes from step 4 on (one per step):
                        # late enough that the previous hp's DVE multiply
                        # chain has drained, early enough (< LAG) that all 4
                        # are emitted before this hp's first row-sum write
                        if i >= 4 and trans_q:
                            trans_q.popleft()()
                        # deficit-rationed filler
                        deficit = 0.0
                        if i < len(kbs):
                            deficit += (2 * w * 0.8333 + 200.0) \
                                - 2 * w * 0.4167
                        if i >= LAG:
                            kb_c, _ = kbs[i - LAG]
                            qbl = max(0, kb_c - 4 * qt) \
                                if mode == "causal" else 0
                            deficit -= 2 * (4 - qbl) * 65 * 0.4167
                        pop_ration(deficit)
                    # normalize: per-partition reciprocal of the row sums,
                    # then one scalar-multiply per (query block, head) into
                    # the [q, e] staging tile, and DMA-transpose each
                    # 128x128 block into ctxt_sb's [e, token] layout
                    # flush deferred transposes before reusing a cq buffer:
                    # the pool only orders against EMITTED readers, so a
                    # still-queued transpose would read clobbered data
                    while trans_q:
                        trans_q.popleft()()
                    rec = norm_sb.tile([128, 8], F32, name="rec", tag="rec")
                    with nc.allow_low_precision(reason="softmax 1/rowsum"):
                        nc.vector.reciprocal(rec[:], rs_ps[:, hp, :])
                    cq = norm_sb.tile([128, 4, 128], BF16, name="cq",
                                      tag="cq")
                    # ONE broadcast tensor_tensor multiply instead of 8
                    # tensor_scalar ops: the reciprocal [128,4,2] broadcasts
                    # along the 64 head dims, cutting the norm chain from
                    # ~2.3us (8 serial DVE ops) to ~0.9us and freeing DVE
                    # throughput for the copies the transposes depend on
                    recv = rec[:].rearrange("p (a b) -> p a b", a=4) \
                        [:, :, :, None].broadcast_to([128, 4, 2, 64])
                    with nc.allow_low_precision(reason="softmax normalize"):
                        nc.vector.tensor_mul(
                            cq[:].rearrange("p a (b d) -> p a b d", b=2),
                            pctx[:], recv)
                    for qb in range(4):
                        # PE transpose back to [e, token] orientation
                        # (stationary load is free; 128 moving columns), then
                        # a VectorE copy into ctxt_sb
                        # defer the transpose: emitted inline it would sit in
                        # the in-order PE stream waiting on the DVE multiply,
                        # delaying the next window's score matmuls behind it.
                        # As a high-priority filler it runs a few steps into
                        # the next window, when the multiply has long drained.
                        def do_transpose(hp=hp, qt=qt, qb=qb, cq=cq):
                            trp = trp_slots[qb]
                            nc.tensor.transpose(trp, cq[:, qb, :], idn_sb[:])
                            nc.vector.tensor_copy(
                                ctxt_sb[hp][:, qt * 512 + 128 * qb:
                                            qt * 512 + 128 * qb + 128],
                                trp,
                            )
                        trans_q.append(do_transpose)
                    # refill the pipeline bubble: PE work emitted after the
                    # norm chain runs while the next head pair's exp warms up.
                    # The transposes just queued above must NOT pop here --
                    # they wait for the DVE multiply chain and would stall the
                    # in-order PE stream; they pop at the next hp's steps 3+.
                    pop_ration(800.0)

            # --- emission ---
            def dma_xpiece(xsb, dview, cb_lo, cb_hi, t_lo, t_hi):
                nc.sync.dma_start(
                    xsb[:].rearrange("p (cb t) -> p cb t", cb=n_cb)
                    [:, cb_lo:cb_hi, t_lo:t_hi],
                    dview[:, cb_lo:cb_hi, t_lo:t_hi],
                )

            def dma_wpiece(wsb, wview, cb_lo, cb_hi):
                nc.sync.dma_start(
                    wsb[:].rearrange("p (cb e) -> p cb e", cb=n_cb)
                    [:, cb_lo:cb_hi],
                    wview[:, cb_lo:cb_hi],
                )

            # PE warmup: the tensor engine runs at half speed until it has
            # been continuously busy for 3us, and the startup is DMA-paced
            # (the PE consumes each arriving piece faster than the next one
            # lands).  Matmuls on a dummy SBUF tile (results land in the psS
            # rotation and are fully overwritten by the first start=True
            # scores matmuls) keep the ramp going; a few are interleaved
            # between the first DMA-gated projection units to bridge the
            # arrival gaps.
            nc.vector.memset(warm_sb[:], 0.0)

            def warm(n):
                for _ in range(n):
                    wps = psS.tile([128, 1024], F32, name="warm_ps",
                                   tag="sc")
                    nc.tensor.matmul(
                        wps[:, 0:256],
                        lhsT=warm_sb[:, 0:128],
                        rhs=warm_sb[:],
                        start=True, stop=True,
                    )

            warm(11)

            # startup: the first projection matmul needs only wk[cb0-3] +
            # xk[cb0-3, first 256 tokens], so split those transfers in half;
            # the PE starts ~2us earlier than with monolithic DMAs.  bias /
            # tri / idn constants ride early: the first bias-add needs bk_sb
            # (a late bias DMA stalls the psMM slot rotation), and tri gates
            # the Pool mask-multiply on window 0's diagonal blocks.
            dma_wpiece(wk_sb, wk_v, 0, 4)
            dma_xpiece(xk_sb, xk_v, 0, 4, 0, 256)
            dma_wpiece(wk_sb, wk_v, 4, 8)
            dma_xpiece(xk_sb, xk_v, 4, 8, 0, 256)
            nc.sync.dma_start(
                bqk_sb[:].rearrange("p (s eb) -> p s eb", s=2),
                bqk.rearrange("s eb p x -> p s (eb x)"))
            nc.sync.dma_start(tri_sb[:], tri[:])
            dma_xpiece(xk_sb, xk_v, 0, n_cb, 256, 512)
            dma_wpiece(wq_sb, wq_v, 0, 4)
            dma_xpiece(xq_sb, xq_v, 0, 4, 0, 256)
            dma_wpiece(wq_sb, wq_v, 4, 8)
            dma_xpiece(xq_sb, xq_v, 4, 8, 0, 256)
            dma_xpiece(xq_sb, xq_v, 0, n_cb, 256, 512)
            nc.sync.dma_start(idn_sb[:], idn[:])
            nc.sync.dma_start(
                wv_sb[:].rearrange("p (cb e) -> p cb e", cb=n_cb), wv_v[:])
            dma_half(xv_sb, xv_v, 0, 0)
            # chunk-1 Q/K ride ahead of the remaining chunk-0 V tokens and
            # wo: window 1's scores consume Q1 at step 0 and K1 at step 4
            # (~25us), and a late-landing K1 head-of-line blocks the whole
            # scheduled PE stream behind it.  Q first: it is needed first.
            dma_one(xq_sb, xq_v, 1)
            dma_one(xk_sb, xk_v, 1)
            dma_half(xv_sb, xv_v, 0, 1)
            # wo here: outproj units become legal filler from window 1 on
            nc.sync.dma_start(
                wo_sb[:].rearrange("p (eb o) -> p eb o", eb=2), wo_v[:])
            dma_half(xv_sb, xv_v, 1, 0)
            dma_half(xv_sb, xv_v, 1, 1)
            # remaining x chunks up-front in priority order: the DMA engine
            # drains this FIFO while the PE works, so later windows'
            # projections never stall on data supply
            for g in range(2, n_qt):
                dma_chunk(g)
                dma_half(xv_sb, xv_v, g, 0)
                dma_half(xv_sb, xv_v, g, 1)

            # group 0 K/Q inline at half-chunk granularity (DMA-paced);
            # interleaved warmups bridge the piece-arrival gaps
            winter = iter((3, 2, 1, 1, 1, 0, 0, 0))
            for xsb, wsb, bias, dst in (
                (xk_sb, wk_sb, bk_sb, kt_sb),
                (xq_sb, wq_sb, bq_sb, qt_sb),
            ):
                for h in range(2):
                    for eb in range(2):
                        qk_unit_half(xsb, wsb, bias, dst, eb, 0, h)
                        warm(next(winter))
            proj_q.extend((f"V{tb}", partial(v_unit, tb)) for tb in range(4))
            qt_order = [0, 1, 2, 3]

            def window_req(qt):
                """Tags that must be emitted before attention(qt) starts."""
                req = []
                hi = qt if mode == "causal" else n_qt
                for g in range(1, hi):  # chunks 1..qt-1 fully
                    req += [f"K{g}e0", f"K{g}e1", f"Q{g}e0", f"Q{g}e1"]
                    req += [f"V{t}" for t in range(4 * g, 4 * g + 4)]
                if mode == "causal" and qt >= 1:
                    req += [f"Q{qt}e0", f"Q{qt}e1"]
                return req

            for wi, qt in enumerate(qt_order):
                if mode == "causal":
                    if wi + 1 < n_qt:
                        units = proj_units(wi + 1)
                        if wi + 1 == n_qt - 1:
                            # queue only Q3 now; hold K3/V3 back as dedicated
                            # window-3 filler (that window has no projection
                            # work of its own and the most exp-bound steps)
                            proj_q.extend(u for u in units
                                          if u[0][0] == "Q")
                            defer_units = [u for u in units
                                           if u[0][0] != "Q"]
                        else:
                            proj_q.extend(units)
                elif wi == 0:  # full mask: window 0 consumes every chunk
                    for g in range(1, n_qt):
                        proj_q.extend(proj_units(g))
                if mode == "causal" and wi == n_qt - 1:
                    proj_q.extend(defer_units)
                force_tags(window_req(qt))
                allow_out[0] = wi >= 1
                out_budget[0] = (0, 1, 3, 999)[wi]
                attention(qt, last_window=(wi == 3))
                out_q.extend(range(4 * qt, 4 * qt + 4))
            while trans_q:  # last window's deferred ctxt transposes
                trans_q.popleft()()
            while proj_q:  # full-mask mode can leave units queued
                tag, fn = proj_q.popleft()
                fn()
            while out_q:
                # tail units split their PSUM->SBUF copies across engines
                outproj_unit(out_q.popleft(), tail=True)

    nc.compile()
    return nc


def get_compiled(mode: str = "causal") -> "bacc.Bacc":
    nc = _compiled_cache.get(mode)
    if nc is None:
        nc = build_nc(mode)
        _compiled_cache[mode] = nc
    return nc


def kernel(query, key, value, mask, Wq, bq, Wk, bk, Wv, bv, Wo, bo):
    query = np.asarray(query, np.float32)
    key = np.asarray(key, np.float32)
    value = np.asarray(value, np.float32)
    mask = np.asarray(mask)
    Wq, bq = np.asarray(Wq, np.float32), np.asarray(bq, np.float32)
    Wk, bk = np.asarray(Wk, np.float32), np.asarray(bk, np.float32)
    Wv, bv = np.asarray(Wv, np.float32), np.asarray(bv, np.float32)
    Wo, bo = np.asarray(Wo, np.float32), np.asarray(bo, np.float32)

    trilm = np.tril(np.ones((S, S), mask.dtype))
    if all(np.array_equal(mask[b], trilm) for b in range(B)):
        mode = "causal"
    elif mask.all():
        mode = "full"
    else:
        raise NotImplementedError("general mask not supported")

    bf = ml_dtypes.bfloat16
    xT = {}
    for nm, arr in (("q", query), ("k", key), ("v", value)):
        xT[nm] = [np.ascontiguousarray(arr[b].T).astype(bf) for b in range(B)]
    WqT = Wq.T.astype(bf)
    WkT = Wk.T.astype(bf)
    WvT = Wv.T.astype(bf)
    WoT = np.ascontiguousarray(Wo.T).astype(bf)
    tri_np = np.where(
        np.arange(128)[:, None] <= np.arange(128)[None, :], 1.0, 0.0
    ).astype(bf)

    in_maps = []
    for c in range(NCORES):
        b, hb = c // 4, c % 4
        es = hb * EPC
        in_maps.append({
            "xq_t": xT["q"][b],
            "xk_t": xT["k"][b],
            "xv_t": xT["v"][b],
            "wq_t": np.ascontiguousarray(WqT[:, es:es + EPC]),
            "wk_t": np.ascontiguousarray(WkT[:, es:es + EPC]),
            "wv_t": np.ascontiguousarray(WvT[:, es:es + EPC]),
            "wo_t": np.ascontiguousarray(WoT[es:es + EPC, :]),
            "bqk2": np.stack([
                bk[es:es + EPC].reshape(2, 128, 1),
                bq[es:es + EPC].reshape(2, 128, 1),
            ]).astype(np.float32),
            "tri": tri_np,
            "idn": np.eye(128, dtype=np.float32).astype(bf),
        })

    nc = get_compiled(mode)
    res = bass_utils.run_bass_kernel_spmd(nc, in_maps, core_ids=list(range(NCORES)))

    const = Wo @ bv + bo
    outf = np.zeros((B, S, D), np.float32)
    for c in range(NCORES):
        outf[c // 4] += res.results[c]["out"].astype(np.float32)
    outf += const[None, None, :]
    return outf



# revision 32
# speedup vs baseline: 1.0186x; 1.0118x over previous
"""Trainium2 Bass kernel for 16-head causal MultiHeadAttention.

Problem: B=2, S=2048, D=1024, H=16 (head_dim 64), causal mask, f32 I/O.

Sharding (8 cores): core c handles batch b = c//4 and head-block hb = c%4
(4 heads = 256 embedding channels). Q/K/V projections are tensor-parallel
column slices; the output projection is tensor-parallel over rows of Wo.T,
so each core emits a partial (S, D) output that the host sums per batch.

Per-core kernel (all matmuls bf16, f32 accumulate):
  1. QT/KT = (Wq_c.T).T @ xT (+bias)   -> (256, 2048) SBUF, e in partitions
  2. V    = xT.T @ Wv_c.T              -> (2048, 256) "V_aug" layout with a
     ones column per head (for softmax row sums)
  3. scoresT[k, q] = KT.T @ QT per 128-key block (both heads of a pair per
     pass), causal blocks only; exp on ScalarE (scale=1/8, no max
     subtraction -- scores are O(5) so exp is safe in f32); the diagonal
     block is masked after exp with a 0/1 triangular multiply on the Pool
     engine (the depth-4 software pipeline hides the extra hop)
  4. ctx in [query, head-dim] orientation: the exp tile is the STATIONARY
     operand (stationary loads are free), so each (key-block, query-block,
     head) costs only 64+1 moving columns -- half the column count of the
     [head-dim, query] orientation. All 8 accumulators share one PSUM bank,
     so the bank is memset once per head pair and the matmuls accumulate
     with start=False (start_tensor_calc's 2KB zero region would clobber
     neighbours). Row sums accumulate as 1-column matmuls into a corner of
     a shared bank.
  5. normalize: per-partition reciprocal of the row sums ([128,8], one DVE
     op), one scalar-multiply per (query block, head), then a PE transpose
     (via identity, staged through bitcast bf16 slices of the shared PSUM
     bank) + VectorE copy into ctxt_sb's [e, token] layout
  6. out_partial = ctxT.T @ Wo_c.T    -> (2048, 1024) bf16 DMA'd out

Scheduling: all x chunk DMAs are issued up-front in priority order;
projection and output-projection work units are interleaved into the
exp-bound attention pipeline as PE filler; the first window's projections
run at half-chunk granularity so the PE starts as soon as the first DMAs
land; output-projection units for earlier windows are held back as filler
for the last window, whose drain units split their PSUM->SBUF copies
across ScalarE/VectorE and DMA each half-tile immediately.

Host: out[b] = sum of the 4 partials + (Wo @ bv + bo).
"""

import sys

for _p in ("/root/.axon_site/_ro/trn_rl_repo", "/opt/trn_rl_repo"):
    if _p not in sys.path:
        sys.path.append(_p)

from collections import deque
from functools import partial

import numpy as np
import ml_dtypes

import concourse.mybir as mybir
import concourse.tile as tile
from concourse import bacc, bass_utils

B, S, D, H = 2, 2048, 1024, 16
HD = D // H  # 64
NCORES = 8
EPC = 256  # embedding channels per core (4 heads)
BF16 = mybir.dt.bfloat16
F32 = mybir.dt.float32

_compiled_cache: dict[str, "bacc.Bacc"] = {}


def _kbs_for(qt: int, mode: str):
    """[(kb, q_lo)] for one 512-wide query tile."""
    if mode == "causal":
        return [(kb, 128 * (kb - 4 * qt) if kb >= 4 * qt else 0)
                for kb in range(4 * qt + 4)]
    return [(kb, 0) for kb in range(S // 128)]


def build_nc(mode: str = "causal") -> "bacc.Bacc":
    nc = bacc.Bacc("TRN2")

    xq = nc.dram_tensor("xq_t", (D, S), BF16, kind="ExternalInput")
    xk = nc.dram_tensor("xk_t", (D, S), BF16, kind="ExternalInput")
    xv = nc.dram_tensor("xv_t", (D, S), BF16, kind="ExternalInput")
    wq = nc.dram_tensor("wq_t", (D, EPC), BF16, kind="ExternalInput")
    wk = nc.dram_tensor("wk_t", (D, EPC), BF16, kind="ExternalInput")
    wv = nc.dram_tensor("wv_t", (D, EPC), BF16, kind="ExternalInput")
    wo = nc.dram_tensor("wo_t", (EPC, D), BF16, kind="ExternalInput")
    bqk = nc.dram_tensor("bqk2", (2, 2, 128, 1), F32, kind="ExternalInput")
    tri = nc.dram_tensor("tri", (128, 128), BF16, kind="ExternalInput")
    idn = nc.dram_tensor("idn", (128, 128), BF16, kind="ExternalInput")
    out = nc.dram_tensor("out", (S, D), BF16, kind="ExternalOutput")

    n_cb = D // 128  # 8 contraction blocks
    n_tb = S // 128  # 16 token blocks
    n_qt = S // 512  # 4 query tiles

    xq_v = xq.rearrange("(cb p) t -> p cb t", p=128)
    xk_v = xk.rearrange("(cb p) t -> p cb t", p=128)
    xv_v = xv.rearrange("(cb p) t -> p cb t", p=128)
    wq_v = wq.rearrange("(cb p) e -> p cb e", p=128)
    wk_v = wk.rearrange("(cb p) e -> p cb e", p=128)
    wv_v = wv.rearrange("(cb p) e -> p cb e", p=128)
    wo_v = wo.rearrange("(eb p) o -> p eb o", p=128)

    with tile.TileContext(nc) as tc:
        with (
            tc.tile_pool(name="consts", bufs=1) as consts,
            tc.tile_pool(name="qkt", bufs=1) as qkt_pool,
            tc.tile_pool(name="vaug", bufs=1) as vaug_pool,
            tc.tile_pool(name="ctxt", bufs=1) as ctxt_pool,
            tc.tile_pool(name="attn_sb", bufs=10) as attn_sb,
            tc.tile_pool(name="norm_sb", bufs=2) as norm_sb,
            tc.tile_pool(name="out_sb", bufs=8) as out_sb,
            # PSUM: 8 banks = psS 2x2 + psC 1x1 + psMM 2x1 + psRS 1x1
            tc.tile_pool(name="psS", bufs=2, space="PSUM") as psS,
            tc.tile_pool(name="psC", bufs=1, space="PSUM") as psC,
            tc.tile_pool(name="psMM", bufs=2, space="PSUM") as psMM,
            tc.tile_pool(name="psRS", bufs=1, space="PSUM") as psRS,
        ):
            # --- resident SBUF tensors ---
            xq_sb = consts.tile([128, n_cb * S], BF16, name="xq_sb")
            xk_sb = consts.tile([128, n_cb * S], BF16, name="xk_sb")
            xv_sb = consts.tile([128, n_cb * S], BF16, name="xv_sb")
            wq_sb = consts.tile([128, n_cb * EPC], BF16, name="wq_sb")
            wk_sb = consts.tile([128, n_cb * EPC], BF16, name="wk_sb")
            wv_sb = consts.tile([128, n_cb * EPC], BF16, name="wv_sb")
            wo_sb = consts.tile([128, 2 * D], BF16, name="wo_sb")
            tri_sb = consts.tile([128, 128], BF16, name="tri_sb")
            idn_sb = consts.tile([128, 128], BF16, name="idn_sb")
            # never written: garbage operand for PE warmup matmuls
            warm_sb = consts.tile([128, 256], BF16, name="warm_sb")
            bqk_sb = consts.tile([128, 4], F32, name="bqk_sb")
            bk_sb = bqk_sb[:, 0:2]
            bq_sb = bqk_sb[:, 2:4]

            qt_sb = [qkt_pool.tile([128, S], BF16, name=f"qt_sb{eb}", tag=f"qt{eb}")
                     for eb in range(2)]
            kt_sb = [qkt_pool.tile([128, S], BF16, name=f"kt_sb{eb}", tag=f"kt{eb}")
                     for eb in range(2)]
            vaug = [vaug_pool.tile([128, 4 * 65], BF16, name=f"vaug{tb}")
                    for tb in range(n_tb)]
            ctxt_sb = [ctxt_pool.tile([128, S], BF16, name=f"ctxt_sb{eb}")
                       for eb in range(2)]
            # one PSUM bank shared by the row-sum accumulators (cols 0-15)
            # and FOUR transpose staging slots (bitcast bf16, cols 64-319).
            # Four slots let all 4 of a head pair's transposes run
            # back-to-back on the in-order PE (with 2 slots, transpose qb+2
            # waits for qb's DVE copy -- a ~620ns PE->DVE->PE ping-pong per
            # block that also stalls every instruction queued behind it)
            rs_big = psRS.tile([128, 512], F32, name="rs_big")
            rs_ps = rs_big[:, 0:16].rearrange("p (a b) -> p a b", a=2)
            trp_slots = [
                rs_big[:, 64 + 64 * i:128 + 64 * i].bitcast(BF16)
                for i in range(4)
            ]

            def xs(t, cb):
                return t[:, cb * S:(cb + 1) * S]

            def ws(t, cb):
                return t[:, cb * EPC:(cb + 1) * EPC]

            def dma_one(xsb, dview, ts_):
                cs = slice(ts_ * 512, (ts_ + 1) * 512)
                nc.sync.dma_start(
                    xsb[:].rearrange("p (cb t) -> p cb t", cb=n_cb)[:, :, cs],
                    dview[:, :, cs],
                )

            def dma_half(xsb, dview, ts_, h):
                """Half-chunk (256-token) DMA: the first V blocks arrive
                ~1.6us earlier, unblocking the first ctx matmuls."""
                cs = slice(ts_ * 512 + h * 256, ts_ * 512 + (h + 1) * 256)
                nc.sync.dma_start(
                    xsb[:].rearrange("p (cb t) -> p cb t", cb=n_cb)[:, :, cs],
                    dview[:, :, cs],
                )

            def dma_chunk(ts_):
                """K and Q chunks now; the V chunk is deferred into the
                filler queue so it does not contend with the scores-critical
                K/Q DMAs at window start."""
                dma_one(xk_sb, xk_v, ts_)
                dma_one(xq_sb, xq_v, ts_)

            def qk_unit(xsb, wsb, bias, dst, eb, ts_):
                cs = slice(ts_ * 512, (ts_ + 1) * 512)
                ps = psMM.tile([128, 512], F32, name="proj_ps", tag="mm")
                for cb in range(n_cb):
                    nc.tensor.matmul(
                        ps[:],
                        lhsT=ws(wsb, cb)[:, eb * 128:(eb + 1) * 128],
                        rhs=xs(xsb, cb)[:, cs],
                        start=(cb == 0), stop=(cb == n_cb - 1),
                    )
                nc.vector.tensor_scalar_add(dst[eb][:, cs], ps[:],
                                            bias[:, eb:eb + 1])

            def qk_unit_half(xsb, wsb, bias, dst, eb, ts_, h):
                """256-token projection unit: paired with half-chunk DMAs at
                startup so the PE starts ~3us earlier."""
                cs = slice(ts_ * 512 + h * 256, ts_ * 512 + (h + 1) * 256)
                ps = psMM.tile([128, 512], F32, name="proj_ps", tag="mm")
                for cb in range(n_cb):
                    nc.tensor.matmul(
                        ps[:, 0:256],
                        lhsT=ws(wsb, cb)[:, eb * 128:(eb + 1) * 128],
                        rhs=xs(xsb, cb)[:, cs],
                        start=(cb == 0), stop=(cb == n_cb - 1),
                    )
                nc.vector.tensor_scalar_add(dst[eb][:, cs], ps[:, 0:256],
                                            bias[:, eb:eb + 1])

            def v_unit(tb):
                ps = psMM.tile([128, EPC], F32, name="v_ps", tag="mm")
                for cb in range(n_cb):
                    nc.tensor.matmul(
                        ps[:],
                        lhsT=xs(xv_sb, cb)[:, tb * 128:(tb + 1) * 128],
                        rhs=ws(wv_sb, cb),
                        start=(cb == 0), stop=(cb == n_cb - 1),
                    )
                va = vaug[tb].rearrange("p (h x) -> p h x", h=4)
                nc.vector.memset(va[:, :, 64:65], 1.0)
                nc.vector.tensor_copy(va[:, :, 0:64],
                                      ps.rearrange("p (h d) -> p h d", h=4))

            def proj_units(ts_):
                """Tagged units for chunk ts_.  K before Q for chunks whose K
                is needed early in their own window; for the last chunk Q
                leads because window 3 needs Q3 at step 0 while K3/V3 can be
                deferred into the window as filler (K3 read from step 12, V3
                from step 19)."""
                kq = [
                    (f"K{ts_}e{eb}",
                     partial(qk_unit, xk_sb, wk_sb, bk_sb, kt_sb, eb, ts_))
                    for eb in range(2)
                ]
                qq = [
                    (f"Q{ts_}e{eb}",
                     partial(qk_unit, xq_sb, wq_sb, bq_sb, qt_sb, eb, ts_))
                    for eb in range(2)
                ]
                units = qq + kq if ts_ == n_qt - 1 else kq + qq
                for tb in range(4 * ts_, 4 * ts_ + 4):
                    units.append((f"V{tb}", partial(v_unit, tb)))
                return units

            def outproj_unit(tb, tail=False):
                ot = out_sb.tile([128, D], BF16, name="out_t", tag="ot")
                if tail:
                    # drain path: alternate the PSUM->SBUF copies across
                    # ScalarE/VectorE and DMA each half as soon as it lands
                    for nb in range(2):
                        po = psMM.tile([128, 512], F32, name="out_ps",
                                       tag="mm")
                        for eb in range(2):
                            nc.tensor.matmul(
                                po[:],
                                lhsT=ctxt_sb[eb][:, tb * 128:(tb + 1) * 128],
                                rhs=wo_sb[:, eb * D + nb * 512:
                                          eb * D + (nb + 1) * 512],
                                start=(eb == 0), stop=(eb == 1),
                            )
                        if nb == 0:
                            nc.scalar.copy(ot[:, 0:512], po[:])
                        else:
                            nc.vector.tensor_copy(ot[:, 512:1024], po[:])
                        nc.sync.dma_start(
                            out[tb * 128:(tb + 1) * 128,
                                nb * 512:(nb + 1) * 512],
                            ot[:, nb * 512:(nb + 1) * 512])
                    return
                for nb in range(2):
                    po = psMM.tile([128, 512], F32, name="out_ps", tag="mm")
                    for eb in range(2):
                        nc.tensor.matmul(
                            po[:],
                            lhsT=ctxt_sb[eb][:, tb * 128:(tb + 1) * 128],
                            rhs=wo_sb[:, eb * D + nb * 512:
                                      eb * D + (nb + 1) * 512],
                            start=(eb == 0), stop=(eb == 1),
                        )
                    nc.vector.tensor_copy(ot[:, nb * 512:(nb + 1) * 512],
                                          po[:])
                nc.sync.dma_start(out[tb * 128:(tb + 1) * 128, :], ot[:])

            proj_q = deque()  # (tag, fn) projection units, deadline-scheduled
            emitted_tags = set()
            out_q = deque()  # holds token-block indices
            trans_q = deque()  # deferred ctxt transposes (highest priority)
            allow_out = [False]
            out_budget = [0]  # outproj pops allowed in the current window
            fill_debt = [0.0]  # ns of PE filler the exp pipeline is owed

            # estimated PE-engine cost of one popped filler unit (ns)
            def unit_cost(tag):
                return 1707.0 if tag[0] in "KQ" else 854.0

            def pop_ration(deficit_ns):
                """Deficit-based filler: accumulate (exp - PE) time per step
                and emit just enough filler to keep the PE fed.  Uniform
                1-pop-per-step both starves exp-heavy stretches (units are
                854-1707ns vs ~400ns/step deficit, so the reserve drains 2-4x
                too fast) and floods the DVE with PSUM->SBUF copies right
                before the norm chain needs it."""
                fill_debt[0] = max(fill_debt[0] + deficit_ns, -1300.0)
                while fill_debt[0] > 0:
                    if proj_q:
                        tag, fn = proj_q.popleft()
                        emitted_tags.add(tag)
                        fill_debt[0] -= unit_cost(tag)
                        fn()
                    elif out_q and allow_out[0] and out_budget[0] > 0:
                        out_budget[0] -= 1
                        fill_debt[0] -= 854.0
                        outproj_unit(out_q.popleft())
                    else:
                        break

            def force_tags(tags):
                """Emit any still-queued proj units bearing these tags NOW
                (data-dependency deadline), preserving queue order."""
                need = {t for t in tags if t not in emitted_tags}
                if not need:
                    return
                keep = deque()
                while proj_q:
                    tag, fn = proj_q.popleft()
                    if tag in need:
                        emitted_tags.add(tag)
                        fn()
                    else:
                        keep.append((tag, fn))
                proj_q.extend(keep)

            def attention(qt, last_window=False):
                # data-dependency deadlines (causal mode): window qt's scores
                # read K-chunk-qt only from step 4qt, and its ctx reads
                # V-chunk-qt only from step 4qt+LAG -- so those projection
                # units stay queued as filler with a forced-emission deadline
                # a couple of steps before first use.
                dl = {}
                if mode == "causal":
                    if qt >= 1:
                        dl[max(1, 4 * qt - 2)] = [f"K{qt}e{eb}"
                                                  for eb in range(2)]
                    dl[4 * qt + 5] = [f"V{tb}"
                                      for tb in range(4 * qt, 4 * qt + 4)]
                for hp in range(2):  # head pair (heads 2hp, 2hp+1)
                    kbs = _kbs_for(qt, mode)
                    # ctx accumulators in [query, head-dim] orientation: one
                    # PSUM bank holds all 4 query blocks x 2 heads x 64 dims;
                    # row sums accumulate as separate 1-column matmuls into
                    # the rs tile. The exp tile is the STATIONARY operand, so
                    # each (key-block, query-block, head) costs only 65
                    # moving columns instead of ~128.
                    pctx = psC.tile([128, 4, 2, 64], F32, name="pctx",
                                    tag="pc")
                    # multiple accumulators share these banks, so PSUM
                    # start_tensor_calc (2KB zero-region granularity) cannot
                    # be used: zero explicitly and accumulate with
                    # start=False throughout
                    nc.vector.memset(pctx[:], 0.0)
                    nc.vector.memset(rs_ps[:, hp, :], 0.0)
                    ets = {}
                    LAG = 9

                    def last_kb(qb):
                        return 4 * qt + qb if mode == "causal" else n_tb - 1

                    for i in range(len(kbs) + LAG):
                        drain_phase = i >= len(kbs)
                        if hp == 0 and i in dl:
                            force_tags(dl[i])
                        if i < len(kbs):
                            kb, q_lo = kbs[i]
                            w = 512 - q_lo
                            crossing = mode == "causal" and kb >= 4 * qt
                            ps = psS.tile([128, 1024], F32, name="sc_ps",
                                          tag="sc")
                            qs = qt * 512 + q_lo
                            for h2 in range(2):
                                nc.tensor.matmul(
                                    ps[:, 512 * h2 + q_lo:512 * h2 + 512],
                                    lhsT=kt_sb[hp][64 * h2:64 * h2 + 64,
                                                   kb * 128:(kb + 1) * 128],
                                    rhs=qt_sb[hp][64 * h2:64 * h2 + 64,
                                                  qs:qs + w],
                                )
                            et = attn_sb.tile([128, 1024], BF16, name="exp_t",
                                              tag="exp")
                            psg = ps.rearrange("p (g c) -> p g c", g=2)
                            etg = et.rearrange("p (g c) -> p g c", g=2)
                            nc.scalar.activation(
                                etg[:, :, q_lo:512], psg[:, :, q_lo:512],
                                mybir.ActivationFunctionType.Exp,
                                scale=0.125,
                            )
                            if crossing:
                                # zero the masked upper half of the diagonal
                                # block with a 0/1 multiply on the (otherwise
                                # idle) Pool engine; the LAG-deep pipeline
                                # hides the extra hop
                                dg = etg[:, :, q_lo:q_lo + 128]
                                nc.gpsimd.tensor_mul(
                                    dg, dg,
                                    tri_sb[:, None, :].broadcast_to(
                                        [128, 2, 128]),
                                )
                            ets[i] = et
                        if i >= LAG:
                            kb, q_lo = kbs[i - LAG]
                            et = ets.pop(i - LAG)
                            etg = et.rearrange("p (g c) -> p g c", g=2)
                            qb_lo = max(0, kb - 4 * qt) \
                                if mode == "causal" else 0
                            for h2 in range(2):
                                hh = 2 * hp + h2
                                for qb in range(qb_lo, 4):
                                    c0 = 128 * qb  # et cols are absolute
                                    stat = etg[:, h2, c0:c0 + 128]
                                    nc.tensor.matmul(
                                        pctx[:, qb, h2, :],
                                        lhsT=stat,
                                        rhs=vaug[kb][:, 65 * hh:65 * hh + 64],
                                        start=False,
                                        stop=(kb == last_kb(qb)),
                                        skip_group_check=True,
                                    )
                                    nc.tensor.matmul(
                                        rs_ps[:, hp, 2 * qb + h2:
                                              2 * qb + h2 + 1],
                                        lhsT=stat,
                                        rhs=vaug[kb][:, 65 * hh + 64:
                                                     65 * hh + 65],
                                        start=False,
                                        stop=(kb == last_kb(qb)),
                                        skip_group_check=True,
                                    )
                        # deferred transposes from step 4 on (one per step):
                        # late enough that the previous hp's DVE multiply
                        # chain has drained, early enough (< LAG) that all 4
                        # are emitted before this hp's first row-sum write
                        if i >= 4 and trans_q:
                            trans_q.popleft()()
                        # deficit-rationed filler
                        deficit = 0.0
                        if i < len(kbs):
                            deficit += (2 * w * 0.8333 + 200.0) \
                                - 2 * w * 0.4167
                        if i >= LAG:
                            kb_c, _ = kbs[i - LAG]
                            qbl = max(0, kb_c - 4 * qt) \
                                if mode == "causal" else 0
                            deficit -= 2 * (4 - qbl) * 65 * 0.4167
                        pop_ration(deficit)
                    # normalize: per-partition reciprocal of the row sums,
                    # then one scalar-multiply per (query block, head) into
                    # the [q, e] staging tile, and DMA-transpose each
                    # 128x128 block into ctxt_sb's [e, token] layout
                    # flush deferred transposes before reusing a cq buffer:
                    # the pool only orders against EMITTED readers, so a
                    # still-queued transpose would read clobbered data
                    while trans_q:
                        trans_q.popleft()()
                    rec = norm_sb.tile([128, 8], F32, name="rec", tag="rec")
                    with nc.allow_low_precision(reason="softmax 1/rowsum"):
                        nc.vector.reciprocal(rec[:], rs_ps[:, hp, :])
                    cq = norm_sb.tile([128, 4, 128], BF16, name="cq",
                                      tag="cq")
                    # ONE broadcast tensor_tensor multiply instead of 8
                    # tensor_scalar ops: the reciprocal [128,4,2] broadcasts
                    # along the 64 head dims, cutting the norm chain from
                    # ~2.3us (8 serial DVE ops) to ~0.9us and freeing DVE
                    # throughput for the copies the transposes depend on
                    recv = rec[:].rearrange("p (a b) -> p a b", a=4) \
                        [:, :, :, None].broadcast_to([128, 4, 2, 64])
                    with nc.allow_low_precision(reason="softmax normalize"):
                        nc.vector.tensor_mul(
                            cq[:].rearrange("p a (b d) -> p a b d", b=2),
                            pctx[:], recv)
                    for qb in range(4):
                        # PE transpose back to [e, token] orientation
                        # (stationary load is free; 128 moving columns), then
                        # a VectorE copy into ctxt_sb
                        # defer the transpose: emitted inline it would sit in
                        # the in-order PE stream waiting on the DVE multiply,
                        # delaying the next window's score matmuls behind it.
                        # As a high-priority filler it runs a few steps into
                        # the next window, when the multiply has long drained.
                        def do_transpose(hp=hp, qt=qt, qb=qb, cq=cq):
                            trp = trp_slots[qb]
                            nc.tensor.transpose(trp, cq[:, qb, :], idn_sb[:])
                            nc.vector.tensor_copy(
                                ctxt_sb[hp][:, qt * 512 + 128 * qb:
                                            qt * 512 + 128 * qb + 128],
                                trp,
                            )
                        trans_q.append(do_transpose)
                    # refill the pipeline bubble: PE work emitted after the
                    # norm chain runs while the next head pair's exp warms up.
                    # The transposes just queued above must NOT pop here --
                    # they wait for the DVE multiply chain and would stall the
                    # in-order PE stream; they pop at the next hp's steps 3+.
                    pop_ration(800.0)

            # --- emission ---
            def dma_xpiece(xsb, dview, cb_lo, cb_hi, t_lo, t_hi):
                nc.sync.dma_start(
                    xsb[:].rearrange("p (cb t) -> p cb t", cb=n_cb)
                    [:, cb_lo:cb_hi, t_lo:t_hi],
                    dview[:, cb_lo:cb_hi, t_lo:t_hi],
                )

            def dma_wpiece(wsb, wview, cb_lo, cb_hi):
                nc.sync.dma_start(
                    wsb[:].rearrange("p (cb e) -> p cb e", cb=n_cb)
                    [:, cb_lo:cb_hi],
                    wview[:, cb_lo:cb_hi],
                )

            # PE warmup: the tensor engine runs at half speed until it has
            # been continuously busy for 3us, and the startup is DMA-paced
            # (the PE consumes each arriving piece faster than the next one
            # lands).  Matmuls on a dummy SBUF tile (results land in the psS
            # rotation and are fully overwritten by the first start=True
            # scores matmuls) keep the ramp going; a few are interleaved
            # between the first DMA-gated projection units to bridge the
            # arrival gaps.
            nc.vector.memset(warm_sb[:], 0.0)

            def warm(n):
                for _ in range(n):
                    wps = psS.tile([128, 1024], F32, name="warm_ps",
                                   tag="sc")
                    nc.tensor.matmul(
                        wps[:, 0:256],
                        lhsT=warm_sb[:, 0:128],
                        rhs=warm_sb[:],
                        start=True, stop=True,
                    )

            warm(11)

            # startup: the first projection matmul needs only wk[cb0-3] +
            # xk[cb0-3, first 256 tokens], so split those transfers in half;
            # the PE starts ~2us earlier than with monolithic DMAs.  bias /
            # tri / idn constants ride early: the first bias-add needs bk_sb
            # (a late bias DMA stalls the psMM slot rotation), and tri gates
            # the Pool mask-multiply on window 0's diagonal blocks.
            dma_wpiece(wk_sb, wk_v, 0, 4)
            dma_xpiece(xk_sb, xk_v, 0, 4, 0, 256)
            dma_wpiece(wk_sb, wk_v, 4, 8)
            dma_xpiece(xk_sb, xk_v, 4, 8, 0, 256)
            nc.sync.dma_start(
                bqk_sb[:].rearrange("p (s eb) -> p s eb", s=2),
                bqk.rearrange("s eb p x -> p s (eb x)"))
            nc.sync.dma_start(tri_sb[:], tri[:])
            dma_xpiece(xk_sb, xk_v, 0, n_cb, 256, 512)
            dma_wpiece(wq_sb, wq_v, 0, 4)
            dma_xpiece(xq_sb, xq_v, 0, 4, 0, 256)
            dma_wpiece(wq_sb, wq_v, 4, 8)
            dma_xpiece(xq_sb, xq_v, 4, 8, 0, 256)
            dma_xpiece(xq_sb, xq_v, 0, n_cb, 256, 512)
            nc.sync.dma_start(idn_sb[:], idn[:])
            nc.sync.dma_start(
                wv_sb[:].rearrange("p (cb e) -> p cb e", cb=n_cb), wv_v[:])
            dma_half(xv_sb, xv_v, 0, 0)
            dma_half(xv_sb, xv_v, 0, 1)
            # chunk-1 Q/K ahead of wo and chunk-1 V: window 1's scores
            # consume Q1 at step 0 and K1 at step 4 (~25us), and a
            # late-landing K1 head-of-line blocks the whole scheduled PE
            # stream behind it.  Q first: it is needed first.
            dma_one(xq_sb, xq_v, 1)
            dma_one(xk_sb, xk_v, 1)
            # wo here: outproj units become legal filler from window 1 on
            nc.sync.dma_start(
                wo_sb[:].rearrange("p (eb o) -> p eb o", eb=2), wo_v[:])
            dma_half(xv_sb, xv_v, 1, 0)
            dma_half(xv_sb, xv_v, 1, 1)
            # remaining x chunks up-front in priority order: the DMA engine
            # drains this FIFO while the PE works, so later windows'
            # projections never stall on data supply
            for g in range(2, n_qt):
                dma_chunk(g)
                dma_half(xv_sb, xv_v, g, 0)
                dma_half(xv_sb, xv_v, g, 1)

            # group 0 K/Q inline at half-chunk granularity (DMA-paced);
            # interleaved warmups bridge the piece-arrival gaps
            winter = iter((3, 2, 1, 1, 1, 0, 0, 0))
            for xsb, wsb, bias, dst in (
                (xk_sb, wk_sb, bk_sb, kt_sb),
                (xq_sb, wq_sb, bq_sb, qt_sb),
            ):
                for h in range(2):
                    for eb in range(2):
                        qk_unit_half(xsb, wsb, bias, dst, eb, 0, h)
                        warm(next(winter))
            proj_q.extend((f"V{tb}", partial(v_unit, tb)) for tb in range(4))
            qt_order = [0, 1, 2, 3]

            def window_req(qt):
                """Tags that must be emitted before attention(qt) starts."""
                req = []
                hi = qt if mode == "causal" else n_qt
                for g in range(1, hi):  # chunks 1..qt-1 fully
                    req += [f"K{g}e0", f"K{g}e1", f"Q{g}e0", f"Q{g}e1"]
                    req += [f"V{t}" for t in range(4 * g, 4 * g + 4)]
                if mode == "causal" and qt >= 1:
                    req += [f"Q{qt}e0", f"Q{qt}e1"]
                return req

            for wi, qt in enumerate(qt_order):
                if mode == "causal":
                    if wi + 1 < n_qt:
                        units = proj_units(wi + 1)
                        if wi + 1 == n_qt - 1:
                            # queue only Q3 now; hold K3/V3 back as dedicated
                            # window-3 filler (that window has no projection
                            # work of its own and the most exp-bound steps)
                            proj_q.extend(u for u in units
                                          if u[0][0] == "Q")
                            defer_units = [u for u in units
                                           if u[0][0] != "Q"]
                        else:
                            proj_q.extend(units)
                elif wi == 0:  # full mask: window 0 consumes every chunk
                    for g in range(1, n_qt):
                        proj_q.extend(proj_units(g))
                if mode == "causal" and wi == n_qt - 1:
                    proj_q.extend(defer_units)
                force_tags(window_req(qt))
                allow_out[0] = wi >= 1
                out_budget[0] = (0, 1, 3, 999)[wi]
                attention(qt, last_window=(wi == 3))
                out_q.extend(range(4 * qt, 4 * qt + 4))
            while trans_q:  # last window's deferred ctxt transposes
                trans_q.popleft()()
            while proj_q:  # full-mask mode can leave units queued
                tag, fn = proj_q.popleft()
                fn()
            while out_q:
                # tail units split their PSUM->SBUF copies across engines
                outproj_unit(out_q.popleft(), tail=True)

    nc.compile()
    return nc


def get_compiled(mode: str = "causal") -> "bacc.Bacc":
    nc = _compiled_cache.get(mode)
    if nc is None:
        nc = build_nc(mode)
        _compiled_cache[mode] = nc
    return nc


def kernel(query, key, value, mask, Wq, bq, Wk, bk, Wv, bv, Wo, bo):
    query = np.asarray(query, np.float32)
    key = np.asarray(key, np.float32)
    value = np.asarray(value, np.float32)
    mask = np.asarray(mask)
    Wq, bq = np.asarray(Wq, np.float32), np.asarray(bq, np.float32)
    Wk, bk = np.asarray(Wk, np.float32), np.asarray(bk, np.float32)
    Wv, bv = np.asarray(Wv, np.float32), np.asarray(bv, np.float32)
    Wo, bo = np.asarray(Wo, np.float32), np.asarray(bo, np.float32)

    trilm = np.tril(np.ones((S, S), mask.dtype))
    if all(np.array_equal(mask[b], trilm) for b in range(B)):
        mode = "causal"
    elif mask.all():
        mode = "full"
    else:
        raise NotImplementedError("general mask not supported")

    bf = ml_dtypes.bfloat16
    xT = {}
    for nm, arr in (("q", query), ("k", key), ("v", value)):
        xT[nm] = [np.ascontiguousarray(arr[b].T).astype(bf) for b in range(B)]
    WqT = Wq.T.astype(bf)
    WkT = Wk.T.astype(bf)
    WvT = Wv.T.astype(bf)
    WoT = np.ascontiguousarray(Wo.T).astype(bf)
    tri_np = np.where(
        np.arange(128)[:, None] <= np.arange(128)[None, :], 1.0, 0.0
    ).astype(bf)

    in_maps = []
    for c in range(NCORES):
        b, hb = c // 4, c % 4
        es = hb * EPC
        in_maps.append({
            "xq_t": xT["q"][b],
            "xk_t": xT["k"][b],
            "xv_t": xT["v"][b],
            "wq_t": np.ascontiguousarray(WqT[:, es:es + EPC]),
            "wk_t": np.ascontiguousarray(WkT[:, es:es + EPC]),
            "wv_t": np.ascontiguousarray(WvT[:, es:es + EPC]),
            "wo_t": np.ascontiguousarray(WoT[es:es + EPC, :]),
            "bqk2": np.stack([
                bk[es:es + EPC].reshape(2, 128, 1),
                bq[es:es + EPC].reshape(2, 128, 1),
            ]).astype(np.float32),
            "tri": tri_np,
            "idn": np.eye(128, dtype=np.float32).astype(bf),
        })

    nc = get_compiled(mode)
    res = bass_utils.run_bass_kernel_spmd(nc, in_maps, core_ids=list(range(NCORES)))

    const = Wo @ bv + bo
    outf = np.zeros((B, S, D), np.float32)
    for c in range(NCORES):
        outf[c // 4] += res.results[c]["out"].astype(np.float32)
    outf += const[None, None, :]
    return outf

